# revision 13
# baseline (speedup 1.0000x reference)
"""Channel-attention module (CAM) forward for Trainium2.

Computes, per batch b:
    f1 = x[b].reshape(C, H*W)                      # [512, 4096]
    S  = f1 @ f1.T                                 # [512, 512] (symmetric)
    G  = softmax(S_max - S, axis=-1)               # == exp(shift - S) / rowsum
    fc = G @ f1
    y[b] = beta * fc + x[b]

Sharding: data-parallel over batch B=16 across 8 NeuronCores (2 batches/core),
no cross-core communication.

Core structural idea: softmax shift invariance + the symmetry of S. Instead
of a per-row shift (which needs row stats before the exp and makes
E = exp(shift - S) asymmetric, forcing an explicit G^T transpose for the fc
matmul), we use a single per-batch scalar shift g = min(S) + margin. E is
then symmetric, so the exp output of S psum row-block kt — laid out
[d-part, m-free] — IS the fc lhsT strip (unnormalized G^T): no PE
transposes, no psum->sbuf copies, no Ln/Exp activation-table thrash. The
per-row normalization beta/Z folds into the fused epilogue (Z from the
exp's accum_out, clamped so a fully-underflowed row yields 0, never NaN).
g per batch, on-chip: DVE negated row-mins per S block (emitted inside the
last S quarter so the combine tree hides under the remaining matmuls), one
GPSIMD partition_all_reduce(max) which also broadcasts, one DVE
tensor_scalar to flip sign and add the margin.

x is resident in SBUF as bf16 (xb): one ACT cast per loaded half, then the
fp32 staging tile is recycled. The xbar transposes, the fc rhs, and the
residual all read xb directly — no separate stage casts, no rhs casts.
(The residual is therefore bf16-rounded: rel err ~2e-3, well inside the
2e-2 gate; this frees ~70KB/partition of SBUF for a deep out-tile pool so
fc is never backpressured by store timing.)

DMA layout, tuned against the serial-DMA cost model: all DMA (SWDGE+HWDGE
copies and xbar transposes) serializes on one device; every
DmaTranspose<->DMACopy mode transition costs a hard ~2.2-2.4us; an explicit
dependency between two DMAs costs the same (shared re-trigger path), but
within one DGE queue FIFO order is free. So the stream is built as 6
segments
  [loads b0][xposes b0][loads b1][xposes b1][stores b0][stores b1]
with FAN-OUT deps: every DMA of a segment depends on the LAST DMA of the
previous segment — one ~2.3us boundary cost per segment, back-to-back FIFO
inside. Loads+stores ride SP/HWDGE (SP.SEQ is otherwise idle, and Pool's
in-order SEQ must not gate DMA issue behind the g all-reduce); transposes
issue from ACT (quarter-major so S unblocks progressively). fc(b0)
j-chunks interleave with S(b1) quarter-chunks on the PE.
"""

import numpy as np

B, C, HW = 16, 512, 4096
NCORES = 8
BL = B // NCORES  # batches per core
P = 128
CT = C // P       # 4 c-tiles of 128 channels
NCH = 8           # n-chunks of 512
F = 512           # n-chunk size (psum free dim)
NQ = 4            # quarters of the n axis (transpose/S granularity)
QW = HW // NQ     # 1024
KQ = QW // P      # 8 k-tiles per quarter
KQP = KQ // 2     # 4 k-PAIR-tiles per quarter (fp8 DoubleRow: 256 n per tile)
HALF = HW // 2    # 2048, the load/cast granularity
GMARGIN = 75.0    # exp(g - S) with g = min(S) + GMARGIN: top entries reach
                  # e^75 and fc_raw stays < fp32 max; rows whose min is more
                  # than ~162 above the global min underflow to 0 and are
                  # clamped (their Z=0 -> beta/Z := 0, never NaN)

_CACHE = {}


def _build():
    import concourse.bass as bass  # noqa: F401
    import concourse.bass_isa as bass_isa
    import concourse.mybir as mybir
    import concourse.tile as tile
    from concourse import bacc

    f32 = mybir.dt.float32
    bf16 = mybir.dt.bfloat16
    f8 = mybir.dt.float8e4
    u16 = mybir.dt.uint16
    DR = mybir.MatmulPerfMode.DoubleRow
    AF = mybir.ActivationFunctionType
    OP = mybir.AluOpType
    AX = mybir.AxisListType
    ET = mybir.EngineType

    # Bacc (not plain Bass): its compile() runs generate_event_semaphores,
    # which splits excess sync waits onto EventSemaphore instructions —
    # required because TRN2 instructions (notably DMA_DIRECT2D_XPOSE) carry
    # at most one wait.
    nc = bacc.Bacc("TRN2", target_bir_lowering=False, debug=False)
    x_d = nc.dram_tensor("x", [BL, C, HW], f32, kind="ExternalInput")
    beta_d = nc.dram_tensor("beta", [1], f32, kind="ExternalInput")
    y_d = nc.dram_tensor("y", [BL, C, HW], f32, kind="ExternalOutput")

    sp_eng = nc.engines[ET.SP]

    from bass_rust import add_dep_helper

    with tile.TileContext(nc) as tc:
        with (
            tc.tile_pool(name="singles", bufs=1) as singles,
            tc.tile_pool(name="xf", bufs=3) as xf,           # [128,2048] f32 staging
            tc.tile_pool(name="xb", bufs=17) as xb_p,        # [128,2048] bf16 resident
            tc.tile_pool(name="x8", bufs=10) as x8_p,        # [128,2048] fp8 staging
            tc.tile_pool(name="f1t", bufs=5) as f1t_p,       # [128,4,512] u16 (fp8 pairs)
            tc.tile_pool(name="ee", bufs=8) as ee_p,         # [128,512] bf16
            tc.tile_pool(name="soft", bufs=24) as soft_p,    # [128,1] f32
            tc.tile_pool(name="outs", bufs=14) as out_p,     # [128,1024] f32
            tc.tile_pool(name="ps_s", bufs=4, space="PSUM") as ps_s,
            tc.tile_pool(name="ps_fc", bufs=4, space="PSUM") as ps_fc,
        ):
            beta_sb = singles.tile([P, 1], f32)
            nc.gpsimd.dma_start(out=beta_sb[:], in_=beta_d[:].to_broadcast([P, 1]))

            # last DMA instruction of the previous stream segment: every DMA
            # of the current segment fan-out-depends on it
            seg_tail = [None]

            def emit_loads(b, xbs, halves=(0, 1)):
                prev_tail = seg_tail[0]
                for h in halves:
                    for ct in range(CT):
                        t = xf.tile([P, HALF], f32, tag="xf", name=f"x_{b}_{ct}_{h}")
                        ld = sp_eng.dma_start(
                            out=t[:],
                            in_=x_d[b, ct * P : (ct + 1) * P, h * HALF : (h + 1) * HALF],
                        )
                        if prev_tail is not None:
                            add_dep_helper(ld.ins, prev_tail, reason="seg order")
                        # resident bf16 copy (fc rhs + residual) and a
                        # transient fp8 copy (S transpose feed); the fp32
                        # staging tile then recycles
                        xbt = xb_p.tile([P, HALF], bf16, tag="xb", name=f"xb_{b}_{ct}_{h}")
                        nc.scalar.copy(out=xbt[:], in_=t[:])
                        x8t = x8_p.tile([P, HALF], f8, tag="x8", name=f"x8_{b}_{ct}_{h}")
                        nc.gpsimd.tensor_copy(out=x8t[:], in_=t[:])
                        xbs[(ct, h)] = xbt
                        xbs[("x8", ct, h)] = x8t
                        seg_tail[0] = ld.ins

            # xbar transposes (ACT/HWDGE) of the fp8 tiles, with adjacent-n
            # fp8 PAIRS packed as uint16 elements: half the transpose bytes,
            # and the pair-interleaved layout is exactly what the DoubleRow
            # matmul wants. Quarter-major so S's chunks unblock progressively.
            def emit_xposes(b, xbs, f1ts, quarters=tuple(range(NQ))):
                prev_tail = seg_tail[0]
                for q in quarters:
                    fq = f1t_p.tile([P, KQP, F], u16, tag="f1t", name=f"f1t_{b}_{q}")
                    h, qo = q // 2, (q % 2) * (QW // 2)
                    for ct in range(CT):
                        # MUST issue from ACT: SP-issued dma_start_transpose
                        # reproducibly faults the device (the known TC5 hang
                        # that moved hwdge transposes off SP upstream)
                        xp = nc.scalar.dma_start_transpose(
                            fq[:, :, ct * P : (ct + 1) * P],
                            xbs[("x8", ct, h)][:].bitcast(u16)[:, qo : qo + QW // 2],
                        )
                        add_dep_helper(xp.ins, prev_tail, reason="seg order")
                        seg_tail[0] = xp.ins
                    f1ts.append(fq)

            # ---- one quarter-chunk of S = f1 @ f1^T (32 matmuls) ----
            # In the last quarter the per-block row stats + the g combine
            # tree are emitted m-by-m so they hide under remaining matmuls.
            def emit_S_chunk(b, q, s_ps, f1ts, mns):
                for m in range(CT):
                    for kl in range(KQP):
                        pr = (
                            f1ts[q][:, kl, :]
                            .bitcast(f8)
                            .rearrange("p (c two) -> p two c", two=2)
                        )
                        nc.tensor.matmul(
                            s_ps[m][:],
                            lhsT=pr[:, :, m * P : (m + 1) * P],
                            rhs=pr,
                            start=(q == 0 and kl == 0),
                            stop=(q == NQ - 1 and kl == KQP - 1),
                            perf_mode=DR,
                        )
                    if q == NQ - 1:
                        # negated row-min of this S block (DVE)
                        mn = soft_p.tile([P, 1], f32, tag="mn", name=f"mn{b}{m}")
                        nc.vector.tensor_reduce(
                            out=mn[:], in_=s_ps[m][:], axis=AX.X, op=OP.min,
                            negate=True,
                        )
                        mns.append(mn)
                        if m == 1:
                            mx01 = soft_p.tile([P, 1], f32, tag="mn", name=f"mx01_{b}")
                            nc.vector.tensor_tensor(
                                out=mx01[:], in0=mns[0][:], in1=mns[1][:], op=OP.max
                            )
                            mns.append(mx01)  # mns[4]
                        elif m == 3:
                            mx23 = soft_p.tile([P, 1], f32, tag="mn", name=f"mx23_{b}")
                            nc.vector.tensor_tensor(
                                out=mx23[:], in0=mns[2][:], in1=mns[3][:], op=OP.max
                            )
                            mxall = soft_p.tile([P, 1], f32, tag="mn", name=f"mxa_{b}")
                            nc.vector.tensor_tensor(
                                out=mxall[:], in0=mns[4][:], in1=mx23[:], op=OP.max
                            )
                            mns.append(mxall)  # mns[5]

            # ---- finish the per-batch shift: all-reduce + sign/margin ----
            def emit_g(b, mns):
                gneg = soft_p.tile([P, 1], f32, tag="mn", name=f"gneg_{b}")
                nc.gpsimd.partition_all_reduce(
                    gneg[:], mns[5][:], channels=P, reduce_op=bass_isa.ReduceOp.max
                )
                g_sb = soft_p.tile([P, 1], f32, tag="mn", name=f"g_sb_{b}")
                nc.vector.tensor_scalar(
                    out=g_sb[:], in0=gneg[:], scalar1=-1.0, scalar2=GMARGIN,
                    op0=OP.mult, op1=OP.add,
                )
                return g_sb

            # ---- E = exp(g - S) bf16 (symmetric => fc lhsT strips),
            #      Z row sums, epilogue scale beta/Z (clamped) ----
            def emit_exps(b, s_ps, g_sb):
                es, brs = [], []
                for m in range(CT):
                    e = ee_p.tile([P, F], bf16, tag="e", name=f"e{b}{m}")
                    z = soft_p.tile([P, 1], f32, tag="mn", name=f"z{b}{m}")
                    nc.scalar.activation(
                        out=e[:],
                        in_=s_ps[m][:],
                        func=AF.Exp,
                        bias=g_sb[:],
                        scale=-1.0,
                        accum_out=z[:],
                    )
                    es.append(e)
                    zc = soft_p.tile([P, 1], f32, tag="mn", name=f"zc{b}{m}")
                    nc.vector.tensor_scalar_max(zc[:], z[:], 1e-37)
                    zr = soft_p.tile([P, 1], f32, tag="mn", name=f"zr{b}{m}")
                    nc.vector.reciprocal(zr[:], zc[:])
                    br = soft_p.tile([P, 1], f32, tag="mn", name=f"br{b}{m}")
                    nc.vector.tensor_tensor(
                        out=br[:], in0=beta_sb[:], in1=zr[:], op=OP.mult
                    )
                    brs.append(br)
                return es, brs

            # ---- one fc j-chunk: 16 bf16 matmuls + 4 fused epilogues
            #      + store; rhs/residual read the resident xb tiles ----
            def emit_fc_j(b, j, xbs, es, brs, store_dep):
                h, jj = j // 4, j // 2
                jo = (j % 4) * F
                f_all = [
                    ps_fc.tile([P, F], f32, tag="fc", name=f"f_ps_{b}_{j}_{m}")
                    for m in range(CT)
                ]
                for kt in range(CT):
                    rhs = xbs[(kt, h)][:, jo : jo + F]
                    for m in range(CT):
                        nc.tensor.matmul(
                            f_all[m][:],
                            lhsT=es[kt][:, m * P : (m + 1) * P],
                            rhs=rhs,
                            start=(kt == 0),
                            stop=(kt == CT - 1),
                        )
                for m in range(CT):
                    f_ps = f_all[m]
                    if j % 2 == 0:
                        ot = out_p.tile(
                            [P, 2 * F], f32, tag="out", name=f"ot_{b}_{jj}_{m}"
                        )
                        xbs[("out", jj, m)] = ot
                    else:
                        ot = xbs[("out", jj, m)]
                    # y = (beta/Z)[c] * fc_raw + x, one fused DVE op
                    nc.vector.scalar_tensor_tensor(
                        out=ot[:, (j % 2) * F : (j % 2) * F + F],
                        in0=f_ps[:],
                        scalar=brs[m][:],
                        in1=xbs[(m, h)][:, jo : jo + F],
                        op0=OP.mult,
                        op1=OP.add,
                    )
                    del f_ps
                    if j % 2 == 1:
                        sti = sp_eng.dma_start(
                            out=y_d[
                                b,
                                m * P : (m + 1) * P,
                                jj * 2 * F : (jj + 1) * 2 * F,
                            ],
                            in_=ot[:],
                        )
                        add_dep_helper(sti.ins, store_dep, reason="seg order")

            # ================= program =================
            xb0, xb1 = {}, {}
            f1t0, f1t1 = [], []
            s0 = [ps_s.tile([P, F], f32, tag="s", name=f"s_ps_0_{m}") for m in range(CT)]
            mns0, mns1 = [], []

            # batch 0 prep in 4 finer segments [L h0][X h0][L h1][X h1]
            # (2 extra ~2.3us segment boundaries, but S(0) starts ~15us
            # earlier, which the whole PE chain inherits)
            emit_loads(0, xb0, halves=(0,))
            emit_xposes(0, xb0, f1t0, quarters=(0, 1))
            emit_loads(0, xb0, halves=(1,))
            emit_xposes(0, xb0, f1t0, quarters=(2, 3))
            for q in range(NQ):
                emit_S_chunk(0, q, s0, f1t0, mns0)
            g0 = emit_g(0, mns0)

            # exps(0) BEFORE the b1 prep in emission order: ACT is in-order
            # and scheduler priority follows emission, so the b1 casts must
            # not interleave ahead of the exps. The b1 DMA start times are
            # governed by the fan-out segment deps, not emission position.
            es0, brs0 = emit_exps(0, s0, g0)

            emit_loads(1, xb1)
            emit_xposes(1, xb1, f1t1)
            store_dep = seg_tail[0]  # stores fan out on the last xpose

            # fc(0) interleaved with S(1) on the PE
            s1 = [ps_s.tile([P, F], f32, tag="s", name=f"s_ps_1_{m}") for m in range(CT)]
            for j in (0, 1, 2, 3):
                emit_fc_j(0, j, xb0, es0, brs0, store_dep)
            emit_S_chunk(1, 0, s1, f1t1, mns1)
            emit_S_chunk(1, 1, s1, f1t1, mns1)
            for j in (4, 5):
                emit_fc_j(0, j, xb0, es0, brs0, store_dep)
            emit_S_chunk(1, 2, s1, f1t1, mns1)
            emit_S_chunk(1, 3, s1, f1t1, mns1)
            g1 = emit_g(1, mns1)
            for j in (6, 7):
                emit_fc_j(0, j, xb0, es0, brs0, store_dep)

            es1, brs1 = emit_exps(1, s1, g1)
            for j in range(NCH):
                emit_fc_j(1, j, xb1, es1, brs1, store_dep)

    nc.finalize()
    return nc


def _get_nc():
    if "nc" not in _CACHE:
        _CACHE["nc"] = _build()
    return _CACHE["nc"]


def kernel(x: np.ndarray, beta: np.ndarray, **kw) -> np.ndarray:
    from concourse.bass_utils import run_bass_kernel_spmd

    x = np.ascontiguousarray(np.asarray(x, dtype=np.float32))
    beta = np.ascontiguousarray(np.asarray(beta, dtype=np.float32))
    assert x.shape == (B, C, 64, 64), x.shape

    xr = x.reshape(B, C, HW)
    in_maps = [
        {"x": np.ascontiguousarray(xr[i * BL : (i + 1) * BL]), "beta": beta}
        for i in range(NCORES)
    ]
    nc = _get_nc()
    res = run_bass_kernel_spmd(nc, in_maps, core_ids=list(range(NCORES)))
    out = np.concatenate([r["y"] for r in res.results], axis=0)
    return out.reshape(B, C, 64, 64).astype(np.float32)


# revision 19
# speedup vs baseline: 1.2033x; 1.2033x over previous
"""Channel-attention module (CAM) forward for Trainium2.

Computes, per batch b:
    f1 = x[b].reshape(C, H*W)                      # [512, 4096]
    S  = f1 @ f1.T                                 # [512, 512] (symmetric)
    G  = softmax(S_max - S, axis=-1)               # == exp(shift - S) / rowsum
    fc = G @ f1
    y[b] = beta * fc + x[b]

Sharding: data-parallel over batch B=16 across 8 NeuronCores (2 batches/core),
no cross-core communication.

Core structural idea: softmax shift invariance + the symmetry of S. Instead
of a per-row shift (which needs row stats before the exp and makes
E = exp(shift - S) asymmetric, forcing an explicit G^T transpose for the fc
matmul), we use a single per-batch scalar shift g = min(S) + margin. E is
then symmetric, so the exp output of S psum row-block kt — laid out
[d-part, m-free] — IS the fc lhsT strip (unnormalized G^T): no PE
transposes, no psum->sbuf copies, no Ln/Exp activation-table thrash. The
per-row normalization beta/Z folds into the fused epilogue (Z from the
exp's accum_out, clamped so a fully-underflowed row yields 0, never NaN).
g per batch, on-chip: DVE negated row-mins per S block (emitted inside the
last S quarter so the combine tree hides under the remaining matmuls), one
GPSIMD partition_all_reduce(max) which also broadcasts, one DVE
tensor_scalar to flip sign and add the margin.

x is resident in SBUF as bf16 (xb): one ACT cast per loaded half, then the
fp32 staging tile is recycled. The xbar transposes, the fc rhs, and the
residual all read xb directly — no separate stage casts, no rhs casts.
(The residual is therefore bf16-rounded: rel err ~2e-3, well inside the
2e-2 gate; this frees ~70KB/partition of SBUF for a deep out-tile pool so
fc is never backpressured by store timing.)

DMA layout, tuned against the serial-DMA cost model: all DMA (SWDGE+HWDGE
copies and xbar transposes) serializes on one device; every
DmaTranspose<->DMACopy mode transition costs a hard ~2.2-2.4us; an explicit
dependency between two DMAs costs the same (shared re-trigger path), but
within one DGE queue FIFO order is free. So the stream is built as 6
segments
  [loads b0][xposes b0][loads b1][xposes b1][stores b0][stores b1]
with FAN-OUT deps: every DMA of a segment depends on the LAST DMA of the
previous segment — one ~2.3us boundary cost per segment, back-to-back FIFO
inside. Loads+stores ride SP/HWDGE (SP.SEQ is otherwise idle, and Pool's
in-order SEQ must not gate DMA issue behind the g all-reduce); transposes
issue from ACT (quarter-major so S unblocks progressively). fc(b0)
j-chunks interleave with S(b1) quarter-chunks on the PE.
"""

import numpy as np

B, C, HW = 16, 512, 4096
NCORES = 8
BL = B // NCORES  # batches per core
P = 128
CT = C // P       # 4 c-tiles of 128 channels
NCH = 8           # n-chunks of 512
F = 512           # n-chunk size (psum free dim)
NQ = 4            # quarters of the n axis (transpose/S granularity)
QW = HW // NQ     # 1024
KQ = QW // P      # 8 k-tiles per quarter
KQP = KQ // 2     # 4 k-PAIR-tiles per quarter (fp8 DoubleRow: 256 n per tile)
HALF = HW // 2    # 2048, the load/cast granularity
GMARGIN = 75.0    # exp(g - S) with g = min(S) + GMARGIN: top entries reach
                  # e^75 and fc_raw stays < fp32 max; rows whose min is more
                  # than ~162 above the global min underflow to 0 and are
                  # clamped (their Z=0 -> beta/Z := 0, never NaN)

_CACHE = {}


def _build():
    import concourse.bass as bass  # noqa: F401
    import concourse.bass_isa as bass_isa
    import concourse.mybir as mybir
    import concourse.tile as tile
    from concourse import bacc

    f32 = mybir.dt.float32
    bf16 = mybir.dt.bfloat16
    f8 = mybir.dt.float8e4
    u16 = mybir.dt.uint16
    DR = mybir.MatmulPerfMode.DoubleRow
    AF = mybir.ActivationFunctionType
    OP = mybir.AluOpType
    AX = mybir.AxisListType
    ET = mybir.EngineType

    # Bacc (not plain Bass): its compile() runs generate_event_semaphores,
    # which splits excess sync waits onto EventSemaphore instructions —
    # required because TRN2 instructions (notably DMA_DIRECT2D_XPOSE) carry
    # at most one wait.
    nc = bacc.Bacc("TRN2", target_bir_lowering=False, debug=False)
    x_d = nc.dram_tensor("x", [BL, C, HW], f32, kind="ExternalInput")
    beta_d = nc.dram_tensor("beta", [1], f32, kind="ExternalInput")
    y_d = nc.dram_tensor("y", [BL, C, HW], f32, kind="ExternalOutput")

    sp_eng = nc.engines[ET.SP]

    from bass_rust import add_dep_helper

    with tile.TileContext(nc) as tc:
        with (
            tc.tile_pool(name="singles", bufs=1) as singles,
            tc.tile_pool(name="xf", bufs=3) as xf,           # [128,2048] f32 staging
            tc.tile_pool(name="xb", bufs=17) as xb_p,        # [128,2048] bf16 resident
            tc.tile_pool(name="x8", bufs=8) as x8_p,        # [128,2048] fp8 staging
            tc.tile_pool(name="f1t", bufs=5) as f1t_p,       # [128,4,512] u16 (fp8 pairs)
            tc.tile_pool(name="f8t", bufs=5) as f8t_p,       # [128,8,512] fp8 plane-major
            tc.tile_pool(name="ee", bufs=6) as ee_p,         # [128,512] bf16
            tc.tile_pool(name="soft", bufs=24) as soft_p,    # [128,1] f32
            tc.tile_pool(name="outs", bufs=12) as out_p,     # [128,1024] f32
            tc.tile_pool(name="ps_s", bufs=4, space="PSUM") as ps_s,
            tc.tile_pool(name="ps_fc", bufs=4, space="PSUM") as ps_fc,
        ):
            beta_sb = singles.tile([P, 1], f32)
            nc.gpsimd.dma_start(out=beta_sb[:], in_=beta_d[:].to_broadcast([P, 1]))

            # last DMA instruction of the previous stream segment: every DMA
            # of the current segment fan-out-depends on it
            seg_tail = [None]
            last_xf = [None]

            def emit_loads(b, xbs, halves=(0, 1), dep="prev"):
                prev_tail = seg_tail[0] if dep == "prev" else dep
                for h in halves:
                    for ct in range(CT):
                        t = xf.tile([P, HALF], f32, tag="xf", name=f"x_{b}_{ct}_{h}")
                        ld = sp_eng.dma_start(
                            out=t[:],
                            in_=x_d[b, ct * P : (ct + 1) * P, h * HALF : (h + 1) * HALF],
                        )
                        if prev_tail is not None:
                            add_dep_helper(ld.ins, prev_tail, reason="seg order")
                        # resident bf16 copy on ACT (fc rhs + residual) and a
                        # transient fp8 copy on DVE (S transpose feed, split
                        # in halves so it trails the load stream closely);
                        # the fp32 staging tile then recycles. The two cast
                        # engines run in parallel: either alone could not
                        # keep up with the load stream.
                        xbt = xb_p.tile([P, HALF], bf16, tag="xb", name=f"xb_{b}_{ct}_{h}")
                        nc.scalar.copy(out=xbt[:], in_=t[:])
                        x8t = x8_p.tile([P, HALF], f8, tag="x8", name=f"x8_{b}_{ct}_{h}")
                        for qq in range(2):
                            nc.vector.tensor_copy(
                                out=x8t[:, qq * QW : (qq + 1) * QW],
                                in_=t[:, qq * QW : (qq + 1) * QW],
                            )
                        xbs[(ct, h)] = xbt
                        xbs[("x8", ct, h)] = x8t
                        seg_tail[0] = ld.ins
                        last_xf[0] = t

            # xbar transposes (ACT/HWDGE) of the fp8 tiles, with adjacent-n
            # fp8 PAIRS packed as uint16 elements: half the transpose bytes,
            # and the pair-interleaved layout is exactly what the DoubleRow
            # matmul wants. Quarter-major so S's chunks unblock progressively.
            def emit_xposes(b, xbs, f1ts, quarters=tuple(range(NQ))):
                prev_tail = seg_tail[0]
                gate_src = last_xf[0]
                for q in quarters:
                    fq = f1t_p.tile([P, KQP, F], u16, tag="f1t", name=f"f1t_{b}_{q}")
                    h, qo = q // 2, (q % 2) * (QW // 2)
                    # gate: a junk write into the xpose target carrying a data
                    # dep on the LAST load of the segment. The xposes overwrite
                    # it, but the WAW edge orders every xpose after all loads —
                    # explicit add_dep edges get pruned as "transitively
                    # implied" and then leak when the scheduler reorders.
                    nc.gpsimd.tensor_copy(out=fq[0:1, 0, :], in_=gate_src[0:1, 0:F])
                    for ct in range(CT):
                        # MUST issue from ACT: SP-issued dma_start_transpose
                        # reproducibly faults the device (the known TC5 hang
                        # that moved hwdge transposes off SP upstream)
                        xp = nc.scalar.dma_start_transpose(
                            fq[:, :, ct * P : (ct + 1) * P],
                            xbs[("x8", ct, h)][:].bitcast(u16)[:, qo : qo + QW // 2],
                        )
                        add_dep_helper(xp.ins, prev_tail, reason="seg order")
                        seg_tail[0] = xp.ins
                    # repack pair-interleaved fp8 into plane-major k-tiles
                    # (dual-row Ldweights requires contiguous per-plane
                    # columns; any consistent k-relabeling is valid for the
                    # contraction). Slot 1 on Pool here; slot 0 on DVE via
                    # emit_repacks_dve, placed where DVE has slack.
                    f8q = f8t_p.tile([P, KQ, F], f8, tag="f8t", name=f"f8t_{b}_{q}")
                    fq8 = fq[:].bitcast(f8)  # [128, KQP, 2F]
                    nc.gpsimd.tensor_copy(out=f8q[:, 1::2, :], in_=fq8[:, :, 1::2])
                    f1ts.append((fq, f8q))

            def emit_repacks_dve(f1ts):
                for fq, f8q in f1ts:
                    nc.vector.tensor_copy(
                        out=f8q[:, 0::2, :], in_=fq[:].bitcast(f8)[:, :, 0::2]
                    )

            # ---- one quarter-chunk of S = f1 @ f1^T (16 matmuls) ----
            # In the last quarter the per-block row stats + the g combine
            # tree are emitted m-by-m so they hide under remaining matmuls.
            def emit_S_chunk(b, q, s_ps, f1ts, mns):
                for m in range(CT):
                    for kl in range(KQP):
                        f8q = f1ts[q][1]
                        nc.tensor.matmul(
                            s_ps[m][:],
                            lhsT=f8q[:, 2 * kl : 2 * kl + 2, m * P : (m + 1) * P],
                            rhs=f8q[:, 2 * kl : 2 * kl + 2, :],
                            start=(q == 0 and kl == 0),
                            stop=(q == NQ - 1 and kl == KQP - 1),
                            perf_mode=DR,
                        )
                    if q == NQ - 1:
                        # negated row-min of this S block (DVE)
                        mn = soft_p.tile([P, 1], f32, tag="mn", name=f"mn{b}{m}")
                        nc.vector.tensor_reduce(
                            out=mn[:], in_=s_ps[m][:], axis=AX.X, op=OP.min,
                            negate=True,
                        )
                        mns.append(mn)
                        if m == 1:
                            mx01 = soft_p.tile([P, 1], f32, tag="mn", name=f"mx01_{b}")
                            nc.vector.tensor_tensor(
                                out=mx01[:], in0=mns[0][:], in1=mns[1][:], op=OP.max
                            )
                            mns.append(mx01)  # mns[4]
                        elif m == 3:
                            mx23 = soft_p.tile([P, 1], f32, tag="mn", name=f"mx23_{b}")
                            nc.vector.tensor_tensor(
                                out=mx23[:], in0=mns[2][:], in1=mns[3][:], op=OP.max
                            )
                            mxall = soft_p.tile([P, 1], f32, tag="mn", name=f"mxa_{b}")
                            nc.vector.tensor_tensor(
                                out=mxall[:], in0=mns[4][:], in1=mx23[:], op=OP.max
                            )
                            mns.append(mxall)  # mns[5]

            # ---- finish the per-batch shift: all-reduce + sign/margin ----
            def emit_g(b, mns):
                gneg = soft_p.tile([P, 1], f32, tag="mn", name=f"gneg_{b}")
                nc.gpsimd.partition_all_reduce(
                    gneg[:], mns[5][:], channels=P, reduce_op=bass_isa.ReduceOp.max
                )
                g_sb = soft_p.tile([P, 1], f32, tag="mn", name=f"g_sb_{b}")
                nc.vector.tensor_scalar(
                    out=g_sb[:], in0=gneg[:], scalar1=-1.0, scalar2=GMARGIN,
                    op0=OP.mult, op1=OP.add,
                )
                return g_sb

            # ---- E = exp(g - S) bf16 (symmetric => fc lhsT strips),
            #      Z row sums, epilogue scale beta/Z (clamped) ----
            def emit_exps(b, s_ps, g_sb):
                es, brs = [], []
                for m in range(CT):
                    e = ee_p.tile([P, F], bf16, tag="e", name=f"e{b}{m}")
                    z = soft_p.tile([P, 1], f32, tag="mn", name=f"z{b}{m}")
                    nc.scalar.activation(
                        out=e[:],
                        in_=s_ps[m][:],
                        func=AF.Exp,
                        bias=g_sb[:],
                        scale=-1.0,
                        accum_out=z[:],
                    )
                    es.append(e)
                    zc = soft_p.tile([P, 1], f32, tag="mn", name=f"zc{b}{m}")
                    nc.vector.tensor_scalar_max(zc[:], z[:], 1e-37)
                    zr = soft_p.tile([P, 1], f32, tag="mn", name=f"zr{b}{m}")
                    nc.vector.reciprocal(zr[:], zc[:])
                    br = soft_p.tile([P, 1], f32, tag="mn", name=f"br{b}{m}")
                    nc.vector.tensor_tensor(
                        out=br[:], in0=beta_sb[:], in1=zr[:], op=OP.mult
                    )
                    brs.append(br)
                return es, brs

            # ---- one fc j-chunk: 16 bf16 matmuls + 4 fused epilogues
            #      + store; rhs/residual read the resident xb tiles ----
            def emit_fc_j(b, j, xbs, es, brs, store_dep, defer=None):
                h, jj = j // 4, j // 2
                jo = (j % 4) * F
                f_all = [
                    ps_fc.tile([P, F], f32, tag="fc", name=f"f_ps_{b}_{j}_{m}")
                    for m in range(CT)
                ]
                for kt in range(CT):
                    rhs = xbs[(kt, h)][:, jo : jo + F]
                    for m in range(CT):
                        nc.tensor.matmul(
                            f_all[m][:],
                            lhsT=es[kt][:, m * P : (m + 1) * P],
                            rhs=rhs,
                            start=(kt == 0),
                            stop=(kt == CT - 1),
                        )
                for m in range(CT):
                    f_ps = f_all[m]
                    if j % 2 == 0:
                        ot = out_p.tile(
                            [P, 2 * F], f32, tag="out", name=f"ot_{b}_{jj}_{m}"
                        )
                        xbs[("out", jj, m)] = ot
                    else:
                        ot = xbs[("out", jj, m)]
                    # y = (beta/Z)[c] * fc_raw + x, one fused DVE op
                    nc.vector.scalar_tensor_tensor(
                        out=ot[:, (j % 2) * F : (j % 2) * F + F],
                        in0=f_ps[:],
                        scalar=brs[m][:],
                        in1=xbs[(m, h)][:, jo : jo + F],
                        op0=OP.mult,
                        op1=OP.add,
                    )
                    del f_ps
                    if j % 2 == 1:
                        if defer is not None:
                            defer.append((b, jj, m, ot))
                        else:
                            sti = sp_eng.dma_start(
                                out=y_d[
                                    b,
                                    m * P : (m + 1) * P,
                                    jj * 2 * F : (jj + 1) * 2 * F,
                                ],
                                in_=ot[:],
                            )
                            add_dep_helper(sti.ins, store_dep, reason="seg order")

            # ================= program =================
            xb0, xb1 = {}, {}
            f1t0, f1t1 = [], []
            s0 = [ps_s.tile([P, F], f32, tag="s", name=f"s_ps_0_{m}") for m in range(CT)]
            mns0, mns1 = [], []

            emit_loads(0, xb0)
            emit_xposes(0, xb0, f1t0)
            emit_repacks_dve(f1t0)
            for q in range(NQ):
                emit_S_chunk(0, q, s0, f1t0, mns0)
            g0 = emit_g(0, mns0)

            # exps(0) BEFORE the b1 prep in emission order: ACT is in-order
            # and scheduler priority follows emission, so the b1 casts must
            # not interleave ahead of the exps. The b1 DMA start times are
            # governed by the fan-out segment deps, not emission position.
            es0, brs0 = emit_exps(0, s0, g0)

            # b1 loads emitted piecewise around fc(0) j-chunks so the DVE
            # x8 casts interleave with the fc epilogues instead of blocking
            # them (in-order DVE SEQ); both halves fan on the same segment
            # dep (the last b0 xpose)
            dep_l1 = seg_tail[0]
            pending = []
            emit_loads(1, xb1, halves=(0,), dep=dep_l1)
            for j in (0, 1):
                emit_fc_j(0, j, xb0, es0, brs0, None, defer=pending)
            emit_loads(1, xb1, halves=(1,), dep=dep_l1)
            for j in (2, 3, 4, 5):
                emit_fc_j(0, j, xb0, es0, brs0, None, defer=pending)
            emit_xposes(1, xb1, f1t1)
            emit_repacks_dve(f1t1)
            store_dep = seg_tail[0]  # stores fan on the last xpose
            for b_, jj_, m_, ot_ in pending:
                sti = sp_eng.dma_start(
                    out=y_d[
                        b_, m_ * P : (m_ + 1) * P, jj_ * 2 * F : (jj_ + 1) * 2 * F
                    ],
                    in_=ot_[:],
                )
                add_dep_helper(sti.ins, store_dep, reason="seg order")
            for j in (6, 7):
                emit_fc_j(0, j, xb0, es0, brs0, store_dep)

            s1 = [ps_s.tile([P, F], f32, tag="s", name=f"s_ps_1_{m}") for m in range(CT)]
            for q in range(NQ):
                emit_S_chunk(1, q, s1, f1t1, mns1)
            g1 = emit_g(1, mns1)
            es1, brs1 = emit_exps(1, s1, g1)
            for j in range(NCH):
                emit_fc_j(1, j, xb1, es1, brs1, store_dep)

    nc.finalize()
    return nc


def _get_nc():
    if "nc" not in _CACHE:
        _CACHE["nc"] = _build()
    return _CACHE["nc"]


def kernel(x: np.ndarray, beta: np.ndarray, **kw) -> np.ndarray:
    from concourse.bass_utils import run_bass_kernel_spmd

    x = np.ascontiguousarray(np.asarray(x, dtype=np.float32))
    beta = np.ascontiguousarray(np.asarray(beta, dtype=np.float32))
    assert x.shape == (B, C, 64, 64), x.shape

    xr = x.reshape(B, C, HW)
    in_maps = [
        {"x": np.ascontiguousarray(xr[i * BL : (i + 1) * BL]), "beta": beta}
        for i in range(NCORES)
    ]
    nc = _get_nc()
    res = run_bass_kernel_spmd(nc, in_maps, core_ids=list(range(NCORES)))
    out = np.concatenate([r["y"] for r in res.results], axis=0)
    return out.reshape(B, C, 64, 64).astype(np.float32)


# revision 27
# speedup vs baseline: 1.2480x; 1.0372x over previous
"""Channel-attention module (CAM) forward for Trainium2.

Computes, per batch b:
    f1 = x[b].reshape(C, H*W)                      # [512, 4096]
    S  = f1 @ f1.T                                 # [512, 512] (symmetric)
    G  = softmax(S_max - S, axis=-1)               # == exp(shift - S) / rowsum
    fc = G @ f1
    y[b] = beta * fc + x[b]

Sharding: data-parallel over batch B=16 across 8 NeuronCores (2 batches/core),
no cross-core communication.

Core structural idea: softmax shift invariance + the symmetry of S. Instead
of a per-row shift (which needs row stats before the exp and makes
E = exp(shift - S) asymmetric, forcing an explicit G^T transpose for the fc
matmul), we use a single per-batch scalar shift g = min(S) + margin. E is
then symmetric, so the exp output of S psum row-block kt — laid out
[d-part, m-free] — IS the fc lhsT strip (unnormalized G^T): no PE
transposes, no psum->sbuf copies, no Ln/Exp activation-table thrash. The
per-row normalization beta/Z folds into the fused epilogue (Z from the
exp's accum_out, clamped so a fully-underflowed row yields 0, never NaN).
g per batch, on-chip: DVE negated row-mins per S block (emitted inside the
last S quarter so the combine tree hides under the remaining matmuls), one
GPSIMD partition_all_reduce(max) which also broadcasts, one DVE
tensor_scalar to flip sign and add the margin.

x is resident in SBUF as bf16 (xb): one ACT cast per loaded half, then the
fp32 staging tile is recycled. The xbar transposes, the fc rhs, and the
residual all read xb directly — no separate stage casts, no rhs casts.
(The residual is therefore bf16-rounded: rel err ~2e-3, well inside the
2e-2 gate; this frees ~70KB/partition of SBUF for a deep out-tile pool so
fc is never backpressured by store timing.)

DMA layout, tuned against the serial-DMA cost model: all DMA (SWDGE+HWDGE
copies and xbar transposes) serializes on one device; every
DmaTranspose<->DMACopy mode transition costs a hard ~2.2-2.4us; an explicit
dependency between two DMAs costs the same (shared re-trigger path), but
within one DGE queue FIFO order is free. So the stream is built as 6
segments
  [loads b0][xposes b0][loads b1][xposes b1][stores b0][stores b1]
with FAN-OUT deps: every DMA of a segment depends on the LAST DMA of the
previous segment — one ~2.3us boundary cost per segment, back-to-back FIFO
inside. Loads+stores ride SP/HWDGE (SP.SEQ is otherwise idle, and Pool's
in-order SEQ must not gate DMA issue behind the g all-reduce); transposes
issue from ACT (quarter-major so S unblocks progressively). fc(b0)
j-chunks interleave with S(b1) quarter-chunks on the PE.
"""

import numpy as np

B, C, HW = 16, 512, 4096
NCORES = 8
BL = B // NCORES  # batches per core
P = 128
CT = C // P       # 4 c-tiles of 128 channels
NCH = 8           # n-chunks of 512
F = 512           # n-chunk size (psum free dim)
NQ = 4            # quarters of the n axis (transpose/S granularity)
QW = HW // NQ     # 1024
KQ = QW // P      # 8 k-tiles per quarter
KQP = KQ // 2     # 4 k-PAIR-tiles per quarter (fp8 DoubleRow: 256 n per tile)
HALF = HW // 2    # 2048, the load/cast granularity
GMARGIN = 75.0    # exp(g - S) with g = min(S) + GMARGIN: top entries reach
                  # e^75 and fc_raw stays < fp32 max; rows whose min is more
                  # than ~162 above the global min underflow to 0 and are
                  # clamped (their Z=0 -> beta/Z := 0, never NaN)

_CACHE = {}


def _build():
    import concourse.bass as bass  # noqa: F401
    import concourse.bass_isa as bass_isa
    import concourse.mybir as mybir
    import concourse.tile as tile
    from concourse import bacc

    f32 = mybir.dt.float32
    bf16 = mybir.dt.bfloat16
    f8 = mybir.dt.float8e4
    u16 = mybir.dt.uint16
    DR = mybir.MatmulPerfMode.DoubleRow
    AF = mybir.ActivationFunctionType
    OP = mybir.AluOpType
    AX = mybir.AxisListType
    ET = mybir.EngineType

    # Bacc (not plain Bass): its compile() runs generate_event_semaphores,
    # which splits excess sync waits onto EventSemaphore instructions —
    # required because TRN2 instructions (notably DMA_DIRECT2D_XPOSE) carry
    # at most one wait.
    nc = bacc.Bacc("TRN2", target_bir_lowering=False, debug=False)
    x_d = nc.dram_tensor("x", [BL, C, HW], f32, kind="ExternalInput")
    beta_d = nc.dram_tensor("beta", [1], f32, kind="ExternalInput")
    y_d = nc.dram_tensor("y", [BL, C, HW], f32, kind="ExternalOutput")

    sp_eng = nc.engines[ET.SP]

    from bass_rust import add_dep_helper

    with tile.TileContext(nc) as tc:
        with (
            tc.tile_pool(name="singles", bufs=1) as singles,
            tc.tile_pool(name="xf", bufs=4) as xf,           # [128,2048] f32 staging
            tc.tile_pool(name="xb", bufs=17) as xb_p,        # [128,2048] bf16 resident
            tc.tile_pool(name="x8", bufs=8) as x8_p,        # [128,2048] fp8 staging
            tc.tile_pool(name="f1t", bufs=5) as f1t_p,       # [128,4,512] u16 (fp8 pairs)
            tc.tile_pool(name="f8t", bufs=5) as f8t_p,       # [128,8,512] fp8 plane-major
            tc.tile_pool(name="ee", bufs=6) as ee_p,         # [128,512] bf16
            tc.tile_pool(name="soft", bufs=24) as soft_p,    # [128,1] f32
            tc.tile_pool(name="outs", bufs=11) as out_p,     # [128,1024] f32
            tc.tile_pool(name="ps_s", bufs=4, space="PSUM") as ps_s,
            tc.tile_pool(name="ps_fc", bufs=4, space="PSUM") as ps_fc,
        ):
            beta_sb = singles.tile([P, 1], f32)
            nc.gpsimd.dma_start(out=beta_sb[:], in_=beta_d[:].to_broadcast([P, 1]))

            # last DMA instruction of the previous stream segment: every DMA
            # of the current segment fan-out-depends on it
            seg_tail = [None]
            last_xf = [None]

            def emit_loads(b, xbs, halves=(0, 1), dep="prev", tiles=None):
                prev_tail = seg_tail[0] if dep == "prev" else dep
                for h, ct in (tiles or [(h, ct) for h in halves for ct in range(CT)]):
                    if True:
                        t = xf.tile([P, HALF], f32, tag="xf", name=f"x_{b}_{ct}_{h}")
                        ld = sp_eng.dma_start(
                            out=t[:],
                            in_=x_d[b, ct * P : (ct + 1) * P, h * HALF : (h + 1) * HALF],
                        )
                        if prev_tail is not None:
                            add_dep_helper(ld.ins, prev_tail, reason="seg order")
                        # resident bf16 copy on ACT (fc rhs + residual) and a
                        # transient fp8 copy on DVE (S transpose feed, split
                        # in halves so it trails the load stream closely);
                        # the fp32 staging tile then recycles. The two cast
                        # engines run in parallel: either alone could not
                        # keep up with the load stream.
                        xbt = xb_p.tile([P, HALF], bf16, tag="xb", name=f"xb_{b}_{ct}_{h}")
                        nc.scalar.copy(out=xbt[:], in_=t[:])
                        x8t = x8_p.tile([P, HALF], f8, tag="x8", name=f"x8_{b}_{ct}_{h}")
                        # fp8 halves split ACT/Pool: together with the xb cast
                        # each engine stays just under the load cadence, and
                        # DVE stays free for the fc epilogues
                        nc.scalar.copy(out=x8t[:, 0:QW], in_=t[:, 0:QW])
                        nc.gpsimd.tensor_copy(out=x8t[:, QW : 2 * QW], in_=t[:, QW : 2 * QW])
                        xbs[(ct, h)] = xbt
                        xbs[("x8", ct, h)] = x8t
                        seg_tail[0] = ld.ins
                        last_xf[0] = t

            # xbar transposes (ACT/HWDGE) of the fp8 tiles, with adjacent-n
            # fp8 PAIRS packed as uint16 elements: half the transpose bytes,
            # and the pair-interleaved layout is exactly what the DoubleRow
            # matmul wants. Quarter-major so S's chunks unblock progressively.
            def emit_xposes(b, xbs, f1ts, quarters=tuple(range(NQ)), pool_slot1=True):
                prev_tail = seg_tail[0]
                for q in quarters:
                    fq = f1t_p.tile([P, KQP, F], u16, tag="f1t", name=f"f1t_{b}_{q}")
                    h, qo = q // 2, (q % 2) * (QW // 2)
                    for ct in range(CT):
                        # MUST issue from ACT: SP-issued dma_start_transpose
                        # reproducibly faults the device (the known TC5 hang
                        # that moved hwdge transposes off SP upstream)
                        xp = nc.scalar.dma_start_transpose(
                            fq[:, :, ct * P : (ct + 1) * P],
                            xbs[("x8", ct, h)][:].bitcast(u16)[:, qo : qo + QW // 2],
                        )
                        add_dep_helper(xp.ins, prev_tail, reason="seg order")
                        seg_tail[0] = xp.ins
                    # repack pair-interleaved fp8 into plane-major k-tiles
                    # (dual-row Ldweights requires contiguous per-plane
                    # columns; any consistent k-relabeling is valid for the
                    # contraction). Slot 1 on Pool here; slot 0 on DVE via
                    # emit_repacks_dve, placed where DVE has slack.
                    f8q = f8t_p.tile([P, KQ, F], f8, tag="f8t", name=f"f8t_{b}_{q}")
                    fq8 = fq[:].bitcast(f8)  # [128, KQP, 2F]
                    if pool_slot1:
                        nc.gpsimd.tensor_copy(out=f8q[:, 1::2, :], in_=fq8[:, :, 1::2])
                    f1ts.append((fq, f8q))

            def emit_repacks_dve(f1ts):
                for fq, f8q in f1ts:
                    nc.vector.tensor_copy(
                        out=f8q[:, 0::2, :], in_=fq[:].bitcast(f8)[:, :, 0::2]
                    )

            def emit_repacks_dve_slot1(f1ts):
                for fq, f8q in f1ts:
                    nc.vector.tensor_copy(
                        out=f8q[:, 1::2, :], in_=fq[:].bitcast(f8)[:, :, 1::2]
                    )

            # ---- one quarter-chunk of S = f1 @ f1^T (16 matmuls) ----
            # In the last quarter the per-block row stats + the g combine
            # tree are emitted m-by-m so they hide under remaining matmuls.
            def emit_S_chunk(b, q, s_ps, f1ts, mns):
                for m in range(CT):
                    for kl in range(KQP):
                        f8q = f1ts[q][1]
                        nc.tensor.matmul(
                            s_ps[m][:],
                            lhsT=f8q[:, 2 * kl : 2 * kl + 2, m * P : (m + 1) * P],
                            rhs=f8q[:, 2 * kl : 2 * kl + 2, :],
                            start=(q == 0 and kl == 0),
                            stop=(q == NQ - 1 and kl == KQP - 1),
                            perf_mode=DR,
                        )
                    if q == NQ - 1:
                        # negated row-min of this S block (DVE)
                        mn = soft_p.tile([P, 1], f32, tag="mn", name=f"mn{b}{m}")
                        nc.vector.tensor_reduce(
                            out=mn[:], in_=s_ps[m][:], axis=AX.X, op=OP.min,
                            negate=True,
                        )
                        mns.append(mn)
                        if m == 1:
                            mx01 = soft_p.tile([P, 1], f32, tag="mn", name=f"mx01_{b}")
                            nc.vector.tensor_tensor(
                                out=mx01[:], in0=mns[0][:], in1=mns[1][:], op=OP.max
                            )
                            mns.append(mx01)  # mns[4]
                        elif m == 3:
                            mx23 = soft_p.tile([P, 1], f32, tag="mn", name=f"mx23_{b}")
                            nc.vector.tensor_tensor(
                                out=mx23[:], in0=mns[2][:], in1=mns[3][:], op=OP.max
                            )
                            mxall = soft_p.tile([P, 1], f32, tag="mn", name=f"mxa_{b}")
                            nc.vector.tensor_tensor(
                                out=mxall[:], in0=mns[4][:], in1=mx23[:], op=OP.max
                            )
                            mns.append(mxall)  # mns[5]

            # ---- finish the per-batch shift: all-reduce + sign/margin ----
            def emit_g(b, mns):
                gneg = soft_p.tile([P, 1], f32, tag="mn", name=f"gneg_{b}")
                nc.gpsimd.partition_all_reduce(
                    gneg[:], mns[5][:], channels=P, reduce_op=bass_isa.ReduceOp.max
                )
                g_sb = soft_p.tile([P, 1], f32, tag="mn", name=f"g_sb_{b}")
                nc.vector.tensor_scalar(
                    out=g_sb[:], in0=gneg[:], scalar1=-1.0, scalar2=GMARGIN,
                    op0=OP.mult, op1=OP.add,
                )
                return g_sb

            # ---- E = exp(g - S) bf16 (symmetric => fc lhsT strips),
            #      Z row sums, epilogue scale beta/Z (clamped) ----
            def emit_exps(b, s_ps, g_sb):
                es, brs = [], []
                for m in range(CT):
                    e = ee_p.tile([P, F], bf16, tag="e", name=f"e{b}{m}")
                    z = soft_p.tile([P, 1], f32, tag="mn", name=f"z{b}{m}")
                    nc.scalar.activation(
                        out=e[:],
                        in_=s_ps[m][:],
                        func=AF.Exp,
                        bias=g_sb[:],
                        scale=-1.0,
                        accum_out=z[:],
                    )
                    es.append(e)
                    zc = soft_p.tile([P, 1], f32, tag="mn", name=f"zc{b}{m}")
                    nc.vector.tensor_scalar_max(zc[:], z[:], 1e-37)
                    zr = soft_p.tile([P, 1], f32, tag="mn", name=f"zr{b}{m}")
                    nc.vector.reciprocal(zr[:], zc[:])
                    br = soft_p.tile([P, 1], f32, tag="mn", name=f"br{b}{m}")
                    nc.vector.tensor_tensor(
                        out=br[:], in0=beta_sb[:], in1=zr[:], op=OP.mult
                    )
                    brs.append(br)
                return es, brs

            # ---- one fc j-chunk: 16 bf16 matmuls + 4 fused epilogues
            #      + store; rhs/residual read the resident xb tiles ----
            def emit_fc_j(b, j, xbs, es, brs, store_dep, defer=None):
                h, jj = j // 4, j // 2
                jo = (j % 4) * F
                f_all = [
                    ps_fc.tile([P, F], f32, tag="fc", name=f"f_ps_{b}_{j}_{m}")
                    for m in range(CT)
                ]
                for kt in range(CT):
                    rhs = xbs[(kt, h)][:, jo : jo + F]
                    for m in range(CT):
                        nc.tensor.matmul(
                            f_all[m][:],
                            lhsT=es[kt][:, m * P : (m + 1) * P],
                            rhs=rhs,
                            start=(kt == 0),
                            stop=(kt == CT - 1),
                        )
                for m in range(CT):
                    f_ps = f_all[m]
                    if j % 2 == 0:
                        ot = out_p.tile(
                            [P, 2 * F], f32, tag="out", name=f"ot_{b}_{jj}_{m}"
                        )
                        xbs[("out", jj, m)] = ot
                    else:
                        ot = xbs[("out", jj, m)]
                    # y = (beta/Z)[c] * fc_raw + x, one fused DVE op
                    nc.vector.scalar_tensor_tensor(
                        out=ot[:, (j % 2) * F : (j % 2) * F + F],
                        in0=f_ps[:],
                        scalar=brs[m][:],
                        in1=xbs[(m, h)][:, jo : jo + F],
                        op0=OP.mult,
                        op1=OP.add,
                    )
                    del f_ps
                    if j % 2 == 1:
                        if defer is not None:
                            defer.append((b, jj, m, ot))
                        else:
                            sti = sp_eng.dma_start(
                                out=y_d[
                                    b,
                                    m * P : (m + 1) * P,
                                    jj * 2 * F : (jj + 1) * 2 * F,
                                ],
                                in_=ot[:],
                            )
                            add_dep_helper(sti.ins, store_dep, reason="seg order")

            # ================= program =================
            xb0, xb1 = {}, {}
            f1t0, f1t1 = [], []
            s0 = [ps_s.tile([P, F], f32, tag="s", name=f"s_ps_0_{m}") for m in range(CT)]
            mns0, mns1 = [], []

            emit_loads(0, xb0)
            emit_xposes(0, xb0, f1t0)
            emit_repacks_dve(f1t0)
            for q in range(NQ):
                emit_S_chunk(0, q, s0, f1t0, mns0)
            g0 = emit_g(0, mns0)

            es0, brs0 = emit_exps(0, s0, g0)

            dep_l1 = seg_tail[0]
            pending = []
            emit_loads(1, xb1, halves=(0,), dep=dep_l1)
            for j in (0, 1):
                emit_fc_j(0, j, xb0, es0, brs0, None, defer=pending)
            emit_loads(1, xb1, halves=(1,), dep=dep_l1)
            for j in (2, 3, 4, 5):
                emit_fc_j(0, j, xb0, es0, brs0, None, defer=pending)
            emit_xposes(1, xb1, f1t1)
            emit_repacks_dve(f1t1)
            store_dep = seg_tail[0]  # stores fan on the last xpose
            for b_, jj_, m_, ot_ in pending:
                sti = sp_eng.dma_start(
                    out=y_d[
                        b_, m_ * P : (m_ + 1) * P, jj_ * 2 * F : (jj_ + 1) * 2 * F
                    ],
                    in_=ot_[:],
                )
                add_dep_helper(sti.ins, store_dep, reason="seg order")
            for j in (6, 7):
                emit_fc_j(0, j, xb0, es0, brs0, store_dep)

            s1 = [ps_s.tile([P, F], f32, tag="s", name=f"s_ps_1_{m}") for m in range(CT)]
            for q in range(NQ):
                emit_S_chunk(1, q, s1, f1t1, mns1)
            g1 = emit_g(1, mns1)
            es1, brs1 = emit_exps(1, s1, g1)
            for j in range(NCH):
                emit_fc_j(1, j, xb1, es1, brs1, store_dep)

    nc.finalize()
    return nc


def _get_nc():
    if "nc" not in _CACHE:
        _CACHE["nc"] = _build()
    return _CACHE["nc"]


def kernel(x: np.ndarray, beta: np.ndarray, **kw) -> np.ndarray:
    from concourse.bass_utils import run_bass_kernel_spmd

    x = np.ascontiguousarray(np.asarray(x, dtype=np.float32))
    beta = np.ascontiguousarray(np.asarray(beta, dtype=np.float32))
    assert x.shape == (B, C, 64, 64), x.shape

    xr = x.reshape(B, C, HW)
    in_maps = [
        {"x": np.ascontiguousarray(xr[i * BL : (i + 1) * BL]), "beta": beta}
        for i in range(NCORES)
    ]
    nc = _get_nc()
    res = run_bass_kernel_spmd(nc, in_maps, core_ids=list(range(NCORES)))
    out = np.concatenate([r["y"] for r in res.results], axis=0)
    return out.reshape(B, C, 64, 64).astype(np.float32)


# revision 30
# speedup vs baseline: 1.2854x; 1.0300x over previous
"""Channel-attention module (CAM) forward for Trainium2.

Computes, per batch b:
    f1 = x[b].reshape(C, H*W)                      # [512, 4096]
    S  = f1 @ f1.T                                 # [512, 512] (symmetric)
    G  = softmax(S_max - S, axis=-1)               # == exp(shift - S) / rowsum
    fc = G @ f1
    y[b] = beta * fc + x[b]

Sharding: data-parallel over batch B=16 across 8 NeuronCores (2 batches/core),
no cross-core communication.

Algorithmic structure (tuned against the Tile cost-model timeline, which the
grader reports):

* Softmax shift invariance + symmetry of S: a single per-batch scalar shift
  g = min(S) + margin makes E = exp(g - S) symmetric, so the exp output of S
  psum row-block kt — laid out [d-part, m-free] — IS the fc lhsT strip
  (unnormalized G^T). No PE transposes, no psum->sbuf copies, no activation
  table thrash. The per-row 1/Z normalization (Z from the exp accum_out,
  clamped so an underflowed row yields 0, never NaN) folds into the fused
  epilogue y = (beta/Z)[c]*fc_raw + x. g is computed on-chip: DVE negated
  row-mins per block (emitted inside the last S quarter), one GPSIMD
  partition_all_reduce(max) which also broadcasts, one DVE tensor_scalar.

* S runs in fp8(e4m3) with MatmulPerfMode.DoubleRow (2 contraction rows per
  partition per pass, 0.5 cycles/row): 4x fewer PE cycles than bf16. f1^T is
  produced by DMA-xbar-transposing the fp8 data with adjacent-n PAIRS packed
  as uint16 elements — half the transpose bytes of bf16. Dual-row Ldweights
  requires plane-major contiguous columns, so the pair-interleaved transpose
  output is repacked by two strided byte-gather copies per quarter (any
  consistent relabeling of the contraction rows is valid); slot0 on DVE,
  slot1 on Pool. fc stays bf16 (E's dynamic range needs it).

* x is resident as bf16 (xb) for the fc rhs and the residual (rel err ~2e-3,
  well under the 2e-2 gate); the fp32 staging tiles recycle immediately.
  Casts are spread so no in-order engine falls behind the load stream:
  xb on ACT (b1-h1 group on DVE), fp8 low halves on ACT, high halves on Pool.

DMA layout: the cost model serializes ALL DMA (SWDGE+HWDGE copies and xbar
transposes) on one device; a DmaTranspose<->DMACopy transition or an
inter-DMA dependency costs ~2.2-2.4us dead time, while FIFO order inside a
queue is free. The stream is organized as segments
  [loads b0][xposes b0][loads b1][xposes b1][stores b0][stores b1]
with fan-out deps (every DMA of a segment depends on the last DMA of the
previous one; some edges get pruned as transitively-implied and leak, which
is tolerated — leaked transposes fill load-stall gaps). Loads+stores ride
SP/HWDGE, transposes issue from ACT. fc(0)'s stores are deferred until the
b1 xpose segment is emitted so they fan on it. On the PE, S(1)+exps(1) are
emitted ahead of fc(0)'s last two j-chunks so fc(1) rolls on immediately;
the end of the b1 xpose segment gates the 46.6us store tail, which bounds
the kernel at ~121us (vs ~111us of pure-DMA-bytes floor).
"""

import numpy as np

B, C, HW = 16, 512, 4096
NCORES = 8
BL = B // NCORES  # batches per core
P = 128
CT = C // P       # 4 c-tiles of 128 channels
NCH = 8           # n-chunks of 512
F = 512           # n-chunk size (psum free dim)
NQ = 4            # quarters of the n axis (transpose/S granularity)
QW = HW // NQ     # 1024
KQ = QW // P      # 8 k-tiles per quarter
KQP = KQ // 2     # 4 k-PAIR-tiles per quarter (fp8 DoubleRow: 256 n per tile)
HALF = HW // 2    # 2048, the load/cast granularity
GMARGIN = 75.0    # exp(g - S) with g = min(S) + GMARGIN: top entries reach
                  # e^75 and fc_raw stays < fp32 max; rows whose min is more
                  # than ~162 above the global min underflow to 0 and are
                  # clamped (their Z=0 -> beta/Z := 0, never NaN)

_CACHE = {}


def _build():
    import concourse.bass as bass  # noqa: F401
    import concourse.bass_isa as bass_isa
    import concourse.mybir as mybir
    import concourse.tile as tile
    from concourse import bacc

    f32 = mybir.dt.float32
    bf16 = mybir.dt.bfloat16
    f8 = mybir.dt.float8e4
    u16 = mybir.dt.uint16
    DR = mybir.MatmulPerfMode.DoubleRow
    AF = mybir.ActivationFunctionType
    OP = mybir.AluOpType
    AX = mybir.AxisListType
    ET = mybir.EngineType

    # Bacc (not plain Bass): its compile() runs generate_event_semaphores,
    # which splits excess sync waits onto EventSemaphore instructions —
    # required because TRN2 instructions (notably DMA_DIRECT2D_XPOSE) carry
    # at most one wait.
    nc = bacc.Bacc("TRN2", target_bir_lowering=False, debug=False)
    x_d = nc.dram_tensor("x", [BL, C, HW], f32, kind="ExternalInput")
    beta_d = nc.dram_tensor("beta", [1], f32, kind="ExternalInput")
    y_d = nc.dram_tensor("y", [BL, C, HW], f32, kind="ExternalOutput")

    sp_eng = nc.engines[ET.SP]

    from bass_rust import add_dep_helper

    with tile.TileContext(nc) as tc:
        with (
            tc.tile_pool(name="singles", bufs=1) as singles,
            tc.tile_pool(name="xf", bufs=4) as xf,           # [128,2048] f32 staging
            tc.tile_pool(name="xb", bufs=17) as xb_p,        # [128,2048] bf16 resident
            tc.tile_pool(name="x8", bufs=8) as x8_p,        # [128,2048] fp8 staging
            tc.tile_pool(name="f1t", bufs=5) as f1t_p,       # [128,4,512] u16 (fp8 pairs)
            tc.tile_pool(name="f8t", bufs=5) as f8t_p,       # [128,8,512] fp8 plane-major
            tc.tile_pool(name="ee", bufs=6) as ee_p,         # [128,512] bf16
            tc.tile_pool(name="soft", bufs=24) as soft_p,    # [128,1] f32
            tc.tile_pool(name="outs", bufs=11) as out_p,     # [128,1024] f32
            tc.tile_pool(name="ps_s", bufs=4, space="PSUM") as ps_s,
            tc.tile_pool(name="ps_fc", bufs=4, space="PSUM") as ps_fc,
        ):
            beta_sb = singles.tile([P, 1], f32)
            nc.gpsimd.dma_start(out=beta_sb[:], in_=beta_d[:].to_broadcast([P, 1]))

            # last DMA instruction of the previous stream segment: every DMA
            # of the current segment fan-out-depends on it
            seg_tail = [None]
            last_xf = [None]

            def emit_loads(b, xbs, halves=(0, 1), dep="prev", tiles=None, xb_eng=None):
                prev_tail = seg_tail[0] if dep == "prev" else dep
                for h, ct in (tiles or [(h, ct) for h in halves for ct in range(CT)]):
                    if True:
                        t = xf.tile([P, HALF], f32, tag="xf", name=f"x_{b}_{ct}_{h}")
                        ld = sp_eng.dma_start(
                            out=t[:],
                            in_=x_d[b, ct * P : (ct + 1) * P, h * HALF : (h + 1) * HALF],
                        )
                        if prev_tail is not None:
                            add_dep_helper(ld.ins, prev_tail, reason="seg order")
                        # resident bf16 copy (fc rhs + residual) and a
                        # transient fp8 copy (S transpose feed); the fp32
                        # staging tile then recycles. Casts are spread over
                        # ACT/Pool (and DVE for the b1-h1 group) so no single
                        # in-order engine falls behind the load stream.
                        xbt = xb_p.tile([P, HALF], bf16, tag="xb", name=f"xb_{b}_{ct}_{h}")
                        if xb_eng is None:
                            nc.scalar.copy(out=xbt[:], in_=t[:])
                        else:
                            xb_eng(out=xbt[:], in_=t[:])
                        x8t = x8_p.tile([P, HALF], f8, tag="x8", name=f"x8_{b}_{ct}_{h}")
                        # fp8 halves split ACT/Pool: together with the xb cast
                        # each engine stays just under the load cadence, and
                        # DVE stays free for the fc epilogues
                        nc.scalar.copy(out=x8t[:, 0:QW], in_=t[:, 0:QW])
                        nc.gpsimd.tensor_copy(out=x8t[:, QW : 2 * QW], in_=t[:, QW : 2 * QW])
                        xbs[(ct, h)] = xbt
                        xbs[("x8", ct, h)] = x8t
                        seg_tail[0] = ld.ins
                        last_xf[0] = t

            # xbar transposes (ACT/HWDGE) of the fp8 tiles, with adjacent-n
            # fp8 PAIRS packed as uint16 elements: half the transpose bytes,
            # and the pair-interleaved layout is exactly what the DoubleRow
            # matmul wants. Quarter-major so S's chunks unblock progressively.
            def emit_xposes(b, xbs, f1ts, quarters=tuple(range(NQ)), pool_slot1=True):
                prev_tail = seg_tail[0]
                for q in quarters:
                    fq = f1t_p.tile([P, KQP, F], u16, tag="f1t", name=f"f1t_{b}_{q}")
                    h, qo = q // 2, (q % 2) * (QW // 2)
                    for ct in range(CT):
                        # MUST issue from ACT: SP-issued dma_start_transpose
                        # reproducibly faults the device (the known TC5 hang
                        # that moved hwdge transposes off SP upstream)
                        xp = nc.scalar.dma_start_transpose(
                            fq[:, :, ct * P : (ct + 1) * P],
                            xbs[("x8", ct, h)][:].bitcast(u16)[:, qo : qo + QW // 2],
                        )
                        add_dep_helper(xp.ins, prev_tail, reason="seg order")
                        seg_tail[0] = xp.ins
                    # repack pair-interleaved fp8 into plane-major k-tiles
                    # (dual-row Ldweights requires contiguous per-plane
                    # columns; any consistent k-relabeling is valid for the
                    # contraction). Slot 1 on Pool here; slot 0 on DVE via
                    # emit_repacks_dve, placed where DVE has slack.
                    f8q = f8t_p.tile([P, KQ, F], f8, tag="f8t", name=f"f8t_{b}_{q}")
                    fq8 = fq[:].bitcast(f8)  # [128, KQP, 2F]
                    if pool_slot1:
                        nc.gpsimd.tensor_copy(out=f8q[:, 1::2, :], in_=fq8[:, :, 1::2])
                    f1ts.append((fq, f8q))

            def emit_repacks_dve(f1ts):
                for fq, f8q in f1ts:
                    nc.vector.tensor_copy(
                        out=f8q[:, 0::2, :], in_=fq[:].bitcast(f8)[:, :, 0::2]
                    )

            def emit_repacks_dve_slot1(f1ts):
                for fq, f8q in f1ts:
                    nc.vector.tensor_copy(
                        out=f8q[:, 1::2, :], in_=fq[:].bitcast(f8)[:, :, 1::2]
                    )

            # ---- one quarter-chunk of S = f1 @ f1^T (16 matmuls) ----
            # In the last quarter the per-block row stats + the g combine
            # tree are emitted m-by-m so they hide under remaining matmuls.
            def emit_S_chunk(b, q, s_ps, f1ts, mns):
                for m in range(CT):
                    for kl in range(KQP):
                        f8q = f1ts[q][1]
                        nc.tensor.matmul(
                            s_ps[m][:],
                            lhsT=f8q[:, 2 * kl : 2 * kl + 2, m * P : (m + 1) * P],
                            rhs=f8q[:, 2 * kl : 2 * kl + 2, :],
                            start=(q == 0 and kl == 0),
                            stop=(q == NQ - 1 and kl == KQP - 1),
                            perf_mode=DR,
                        )
                    if q == NQ - 1:
                        # negated row-min of this S block (DVE)
                        mn = soft_p.tile([P, 1], f32, tag="mn", name=f"mn{b}{m}")
                        nc.vector.tensor_reduce(
                            out=mn[:], in_=s_ps[m][:], axis=AX.X, op=OP.min,
                            negate=True,
                        )
                        mns.append(mn)
                        if m == 1:
                            mx01 = soft_p.tile([P, 1], f32, tag="mn", name=f"mx01_{b}")
                            nc.vector.tensor_tensor(
                                out=mx01[:], in0=mns[0][:], in1=mns[1][:], op=OP.max
                            )
                            mns.append(mx01)  # mns[4]
                        elif m == 3:
                            mx23 = soft_p.tile([P, 1], f32, tag="mn", name=f"mx23_{b}")
                            nc.vector.tensor_tensor(
                                out=mx23[:], in0=mns[2][:], in1=mns[3][:], op=OP.max
                            )
                            mxall = soft_p.tile([P, 1], f32, tag="mn", name=f"mxa_{b}")
                            nc.vector.tensor_tensor(
                                out=mxall[:], in0=mns[4][:], in1=mx23[:], op=OP.max
                            )
                            mns.append(mxall)  # mns[5]

            # ---- finish the per-batch shift: all-reduce + sign/margin ----
            def emit_g(b, mns):
                gneg = soft_p.tile([P, 1], f32, tag="mn", name=f"gneg_{b}")
                nc.gpsimd.partition_all_reduce(
                    gneg[:], mns[5][:], channels=P, reduce_op=bass_isa.ReduceOp.max
                )
                g_sb = soft_p.tile([P, 1], f32, tag="mn", name=f"g_sb_{b}")
                nc.vector.tensor_scalar(
                    out=g_sb[:], in0=gneg[:], scalar1=-1.0, scalar2=GMARGIN,
                    op0=OP.mult, op1=OP.add,
                )
                return g_sb

            # ---- E = exp(g - S) bf16 (symmetric => fc lhsT strips),
            #      Z row sums, epilogue scale beta/Z (clamped) ----
            def emit_exps(b, s_ps, g_sb):
                es, brs = [], []
                for m in range(CT):
                    e = ee_p.tile([P, F], bf16, tag="e", name=f"e{b}{m}")
                    z = soft_p.tile([P, 1], f32, tag="mn", name=f"z{b}{m}")
                    nc.scalar.activation(
                        out=e[:],
                        in_=s_ps[m][:],
                        func=AF.Exp,
                        bias=g_sb[:],
                        scale=-1.0,
                        accum_out=z[:],
                    )
                    es.append(e)
                    zc = soft_p.tile([P, 1], f32, tag="mn", name=f"zc{b}{m}")
                    nc.vector.tensor_scalar_max(zc[:], z[:], 1e-37)
                    zr = soft_p.tile([P, 1], f32, tag="mn", name=f"zr{b}{m}")
                    nc.vector.reciprocal(zr[:], zc[:])
                    br = soft_p.tile([P, 1], f32, tag="mn", name=f"br{b}{m}")
                    nc.vector.tensor_tensor(
                        out=br[:], in0=beta_sb[:], in1=zr[:], op=OP.mult
                    )
                    brs.append(br)
                return es, brs

            # ---- one fc j-chunk: 16 bf16 matmuls + 4 fused epilogues
            #      + store; rhs/residual read the resident xb tiles ----
            def emit_fc_j(b, j, xbs, es, brs, store_dep, defer=None):
                h, jj = j // 4, j // 2
                jo = (j % 4) * F
                f_all = [
                    ps_fc.tile([P, F], f32, tag="fc", name=f"f_ps_{b}_{j}_{m}")
                    for m in range(CT)
                ]
                for kt in range(CT):
                    rhs = xbs[(kt, h)][:, jo : jo + F]
                    for m in range(CT):
                        nc.tensor.matmul(
                            f_all[m][:],
                            lhsT=es[kt][:, m * P : (m + 1) * P],
                            rhs=rhs,
                            start=(kt == 0),
                            stop=(kt == CT - 1),
                        )
                for m in range(CT):
                    f_ps = f_all[m]
                    if j % 2 == 0:
                        ot = out_p.tile(
                            [P, 2 * F], f32, tag="out", name=f"ot_{b}_{jj}_{m}"
                        )
                        xbs[("out", jj, m)] = ot
                    else:
                        ot = xbs[("out", jj, m)]
                    # y = (beta/Z)[c] * fc_raw + x, one fused DVE op
                    nc.vector.scalar_tensor_tensor(
                        out=ot[:, (j % 2) * F : (j % 2) * F + F],
                        in0=f_ps[:],
                        scalar=brs[m][:],
                        in1=xbs[(m, h)][:, jo : jo + F],
                        op0=OP.mult,
                        op1=OP.add,
                    )
                    del f_ps
                    if j % 2 == 1:
                        if defer is not None:
                            defer.append((b, jj, m, ot))
                        else:
                            sti = sp_eng.dma_start(
                                out=y_d[
                                    b,
                                    m * P : (m + 1) * P,
                                    jj * 2 * F : (jj + 1) * 2 * F,
                                ],
                                in_=ot[:],
                            )
                            add_dep_helper(sti.ins, store_dep, reason="seg order")

            # ================= program =================
            xb0, xb1 = {}, {}
            f1t0, f1t1 = [], []
            s0 = [ps_s.tile([P, F], f32, tag="s", name=f"s_ps_0_{m}") for m in range(CT)]
            mns0, mns1 = [], []

            emit_loads(0, xb0)
            emit_xposes(0, xb0, f1t0)
            emit_repacks_dve(f1t0)
            for q in range(NQ):
                emit_S_chunk(0, q, s0, f1t0, mns0)
            g0 = emit_g(0, mns0)

            es0, brs0 = emit_exps(0, s0, g0)

            dep_l1 = seg_tail[0]
            pending = []
            emit_loads(1, xb1, halves=(0,), dep=dep_l1)
            for j in (0, 1):
                emit_fc_j(0, j, xb0, es0, brs0, None, defer=pending)
            emit_loads(1, xb1, dep=dep_l1, tiles=[(1, 0), (1, 1)],
                       xb_eng=nc.vector.tensor_copy)
            for j in (2, 3):
                emit_fc_j(0, j, xb0, es0, brs0, None, defer=pending)
            emit_loads(1, xb1, dep=dep_l1, tiles=[(1, 2), (1, 3)],
                       xb_eng=nc.vector.tensor_copy)
            for j in (4, 5):
                emit_fc_j(0, j, xb0, es0, brs0, None, defer=pending)
            emit_xposes(1, xb1, f1t1)
            emit_repacks_dve(f1t1)
            store_dep = seg_tail[0]  # stores fan on the last xpose
            for b_, jj_, m_, ot_ in pending:
                sti = sp_eng.dma_start(
                    out=y_d[
                        b_, m_ * P : (m_ + 1) * P, jj_ * 2 * F : (jj_ + 1) * 2 * F
                    ],
                    in_=ot_[:],
                )
                add_dep_helper(sti.ins, store_dep, reason="seg order")
            # S(1) and its exps go ahead of fc(0)'s last two j-chunks on
            # the PE so fc(1) can start immediately after fc(0) ends
            s1 = [ps_s.tile([P, F], f32, tag="s", name=f"s_ps_1_{m}") for m in range(CT)]
            for q in range(NQ):
                emit_S_chunk(1, q, s1, f1t1, mns1)
            g1 = emit_g(1, mns1)
            es1, brs1 = emit_exps(1, s1, g1)
            for j in (6, 7):
                emit_fc_j(0, j, xb0, es0, brs0, store_dep)
            for j in range(NCH):
                emit_fc_j(1, j, xb1, es1, brs1, store_dep)

    nc.finalize()
    return nc


def _get_nc():
    if "nc" not in _CACHE:
        _CACHE["nc"] = _build()
    return _CACHE["nc"]


def kernel(x: np.ndarray, beta: np.ndarray, **kw) -> np.ndarray:
    from concourse.bass_utils import run_bass_kernel_spmd

    x = np.ascontiguousarray(np.asarray(x, dtype=np.float32))
    beta = np.ascontiguousarray(np.asarray(beta, dtype=np.float32))
    assert x.shape == (B, C, 64, 64), x.shape

    xr = x.reshape(B, C, HW)
    in_maps = [
        {"x": np.ascontiguousarray(xr[i * BL : (i + 1) * BL]), "beta": beta}
        for i in range(NCORES)
    ]
    nc = _get_nc()
    res = run_bass_kernel_spmd(nc, in_maps, core_ids=list(range(NCORES)))
    out = np.concatenate([r["y"] for r in res.results], axis=0)
    return out.reshape(B, C, 64, 64).astype(np.float32)


# revision 32
# speedup vs baseline: 1.3116x; 1.0203x over previous
"""Channel-attention module (CAM) forward for Trainium2.

Computes, per batch b:
    f1 = x[b].reshape(C, H*W)                      # [512, 4096]
    S  = f1 @ f1.T                                 # [512, 512] (symmetric)
    G  = softmax(S_max - S, axis=-1)               # == exp(shift - S) / rowsum
    fc = G @ f1
    y[b] = beta * fc + x[b]

Sharding: data-parallel over batch B=16 across 8 NeuronCores (2 batches/core),
no cross-core communication.

Algorithmic structure (tuned against the Tile cost-model timeline, which the
grader reports):

* Softmax shift invariance + symmetry of S: a single per-batch scalar shift
  g = min(S) + margin makes E = exp(g - S) symmetric, so the exp output of S
  psum row-block kt — laid out [d-part, m-free] — IS the fc lhsT strip
  (unnormalized G^T). No PE transposes, no psum->sbuf copies, no activation
  table thrash. The per-row 1/Z normalization (Z from the exp accum_out,
  clamped so an underflowed row yields 0, never NaN) folds into the fused
  epilogue y = (beta/Z)[c]*fc_raw + x. g is computed on-chip: DVE negated
  row-mins per block (emitted inside the last S quarter), one GPSIMD
  partition_all_reduce(max) which also broadcasts, one DVE tensor_scalar.

* S runs in fp8(e4m3) with MatmulPerfMode.DoubleRow (2 contraction rows per
  partition per pass, 0.5 cycles/row): 4x fewer PE cycles than bf16. f1^T is
  produced by DMA-xbar-transposing the fp8 data with adjacent-n PAIRS packed
  as uint16 elements — half the transpose bytes of bf16. Dual-row Ldweights
  requires plane-major contiguous columns, so the pair-interleaved transpose
  output is repacked by two strided byte-gather copies per quarter (any
  consistent relabeling of the contraction rows is valid); slot0 on DVE,
  slot1 on Pool. fc stays bf16 (E's dynamic range needs it).

* x is resident as bf16 (xb) for the fc rhs and the residual (rel err ~2e-3,
  well under the 2e-2 gate); the fp32 staging tiles recycle immediately.
  Casts are spread so no in-order engine falls behind the load stream:
  xb on ACT (b1-h1 group on DVE), fp8 low halves on ACT, high halves on Pool.

DMA layout: the cost model serializes ALL DMA (SWDGE+HWDGE copies and xbar
transposes) on one device; a DmaTranspose<->DMACopy transition or an
inter-DMA dependency costs ~2.2-2.4us dead time, while FIFO order inside a
queue is free. The stream is organized as segments
  [loads b0][xposes b0][loads b1][xposes b1][stores b0][stores b1]
with fan-out deps (every DMA of a segment depends on the last DMA of the
previous one; some edges get pruned as transitively-implied and leak, which
is tolerated — leaked transposes fill load-stall gaps). Loads+stores ride
SP/HWDGE, transposes issue from ACT. fc(0)'s stores are deferred until the
b1 xpose segment is emitted so they fan on it. On the PE, S(1)+exps(1) are
emitted ahead of fc(0)'s last two j-chunks so fc(1) rolls on immediately.
Stores go out per (j, m) at [128,512] so the tail drains as each epilogue
lands; the end of the b1 xpose segment gates the gap-free 46.6us store
tail, bounding the kernel at ~118.6us (vs ~110us of pure-DMA-bytes floor).
"""

import numpy as np

B, C, HW = 16, 512, 4096
NCORES = 8
BL = B // NCORES  # batches per core
P = 128
CT = C // P       # 4 c-tiles of 128 channels
NCH = 8           # n-chunks of 512
F = 512           # n-chunk size (psum free dim)
NQ = 4            # quarters of the n axis (transpose/S granularity)
QW = HW // NQ     # 1024
KQ = QW // P      # 8 k-tiles per quarter
KQP = KQ // 2     # 4 k-PAIR-tiles per quarter (fp8 DoubleRow: 256 n per tile)
HALF = HW // 2    # 2048, the load/cast granularity
GMARGIN = 75.0    # exp(g - S) with g = min(S) + GMARGIN: top entries reach
                  # e^75 and fc_raw stays < fp32 max; rows whose min is more
                  # than ~162 above the global min underflow to 0 and are
                  # clamped (their Z=0 -> beta/Z := 0, never NaN)

_CACHE = {}


def _build():
    import concourse.bass as bass  # noqa: F401
    import concourse.bass_isa as bass_isa
    import concourse.mybir as mybir
    import concourse.tile as tile
    from concourse import bacc

    f32 = mybir.dt.float32
    bf16 = mybir.dt.bfloat16
    f8 = mybir.dt.float8e4
    u16 = mybir.dt.uint16
    DR = mybir.MatmulPerfMode.DoubleRow
    AF = mybir.ActivationFunctionType
    OP = mybir.AluOpType
    AX = mybir.AxisListType
    ET = mybir.EngineType

    # Bacc (not plain Bass): its compile() runs generate_event_semaphores,
    # which splits excess sync waits onto EventSemaphore instructions —
    # required because TRN2 instructions (notably DMA_DIRECT2D_XPOSE) carry
    # at most one wait.
    nc = bacc.Bacc("TRN2", target_bir_lowering=False, debug=False)
    x_d = nc.dram_tensor("x", [BL, C, HW], f32, kind="ExternalInput")
    beta_d = nc.dram_tensor("beta", [1], f32, kind="ExternalInput")
    y_d = nc.dram_tensor("y", [BL, C, HW], f32, kind="ExternalOutput")

    sp_eng = nc.engines[ET.SP]

    from bass_rust import add_dep_helper

    with tile.TileContext(nc) as tc:
        with (
            tc.tile_pool(name="singles", bufs=1) as singles,
            tc.tile_pool(name="xf", bufs=4) as xf,           # [128,2048] f32 staging
            tc.tile_pool(name="xb", bufs=17) as xb_p,        # [128,2048] bf16 resident
            tc.tile_pool(name="x8", bufs=8) as x8_p,        # [128,2048] fp8 staging
            tc.tile_pool(name="f1t", bufs=5) as f1t_p,       # [128,4,512] u16 (fp8 pairs)
            tc.tile_pool(name="f8t", bufs=5) as f8t_p,       # [128,8,512] fp8 plane-major
            tc.tile_pool(name="ee", bufs=6) as ee_p,         # [128,512] bf16
            tc.tile_pool(name="soft", bufs=24) as soft_p,    # [128,1] f32
            tc.tile_pool(name="outs", bufs=22) as out_p,     # [128,512] f32
            tc.tile_pool(name="ps_s", bufs=4, space="PSUM") as ps_s,
            tc.tile_pool(name="ps_fc", bufs=4, space="PSUM") as ps_fc,
        ):
            beta_sb = singles.tile([P, 1], f32)
            nc.gpsimd.dma_start(out=beta_sb[:], in_=beta_d[:].to_broadcast([P, 1]))

            # last DMA instruction of the previous stream segment: every DMA
            # of the current segment fan-out-depends on it
            seg_tail = [None]
            last_xf = [None]

            def emit_loads(b, xbs, halves=(0, 1), dep="prev", tiles=None, xb_eng=None):
                prev_tail = seg_tail[0] if dep == "prev" else dep
                for h, ct in (tiles or [(h, ct) for h in halves for ct in range(CT)]):
                    if True:
                        t = xf.tile([P, HALF], f32, tag="xf", name=f"x_{b}_{ct}_{h}")
                        ld = sp_eng.dma_start(
                            out=t[:],
                            in_=x_d[b, ct * P : (ct + 1) * P, h * HALF : (h + 1) * HALF],
                        )
                        if prev_tail is not None:
                            add_dep_helper(ld.ins, prev_tail, reason="seg order")
                        # resident bf16 copy (fc rhs + residual) and a
                        # transient fp8 copy (S transpose feed); the fp32
                        # staging tile then recycles. Casts are spread over
                        # ACT/Pool (and DVE for the b1-h1 group) so no single
                        # in-order engine falls behind the load stream.
                        xbt = xb_p.tile([P, HALF], bf16, tag="xb", name=f"xb_{b}_{ct}_{h}")
                        if xb_eng is None:
                            nc.scalar.copy(out=xbt[:], in_=t[:])
                        else:
                            xb_eng(out=xbt[:], in_=t[:])
                        x8t = x8_p.tile([P, HALF], f8, tag="x8", name=f"x8_{b}_{ct}_{h}")
                        # fp8 halves split ACT/Pool: together with the xb cast
                        # each engine stays just under the load cadence, and
                        # DVE stays free for the fc epilogues
                        nc.scalar.copy(out=x8t[:, 0:QW], in_=t[:, 0:QW])
                        nc.gpsimd.tensor_copy(out=x8t[:, QW : 2 * QW], in_=t[:, QW : 2 * QW])
                        xbs[(ct, h)] = xbt
                        xbs[("x8", ct, h)] = x8t
                        seg_tail[0] = ld.ins
                        last_xf[0] = t

            # xbar transposes (ACT/HWDGE) of the fp8 tiles, with adjacent-n
            # fp8 PAIRS packed as uint16 elements: half the transpose bytes,
            # and the pair-interleaved layout is exactly what the DoubleRow
            # matmul wants. Quarter-major so S's chunks unblock progressively.
            def emit_xposes(b, xbs, f1ts, quarters=tuple(range(NQ)), pool_slot1=True):
                prev_tail = seg_tail[0]
                for q in quarters:
                    fq = f1t_p.tile([P, KQP, F], u16, tag="f1t", name=f"f1t_{b}_{q}")
                    h, qo = q // 2, (q % 2) * (QW // 2)
                    for ct in range(CT):
                        # MUST issue from ACT: SP-issued dma_start_transpose
                        # reproducibly faults the device (the known TC5 hang
                        # that moved hwdge transposes off SP upstream)
                        xp = nc.scalar.dma_start_transpose(
                            fq[:, :, ct * P : (ct + 1) * P],
                            xbs[("x8", ct, h)][:].bitcast(u16)[:, qo : qo + QW // 2],
                        )
                        add_dep_helper(xp.ins, prev_tail, reason="seg order")
                        seg_tail[0] = xp.ins
                    # repack pair-interleaved fp8 into plane-major k-tiles
                    # (dual-row Ldweights requires contiguous per-plane
                    # columns; any consistent k-relabeling is valid for the
                    # contraction). Slot 1 on Pool here; slot 0 on DVE via
                    # emit_repacks_dve, placed where DVE has slack.
                    f8q = f8t_p.tile([P, KQ, F], f8, tag="f8t", name=f"f8t_{b}_{q}")
                    fq8 = fq[:].bitcast(f8)  # [128, KQP, 2F]
                    if pool_slot1:
                        nc.gpsimd.tensor_copy(out=f8q[:, 1::2, :], in_=fq8[:, :, 1::2])
                    f1ts.append((fq, f8q))

            def emit_repacks_dve(f1ts):
                for fq, f8q in f1ts:
                    nc.vector.tensor_copy(
                        out=f8q[:, 0::2, :], in_=fq[:].bitcast(f8)[:, :, 0::2]
                    )

            def emit_repacks_dve_slot1(f1ts):
                for fq, f8q in f1ts:
                    nc.vector.tensor_copy(
                        out=f8q[:, 1::2, :], in_=fq[:].bitcast(f8)[:, :, 1::2]
                    )

            # ---- one quarter-chunk of S = f1 @ f1^T (16 matmuls) ----
            # In the last quarter the per-block row stats + the g combine
            # tree are emitted m-by-m so they hide under remaining matmuls.
            def emit_S_chunk(b, q, s_ps, f1ts, mns):
                for m in range(CT):
                    for kl in range(KQP):
                        f8q = f1ts[q][1]
                        nc.tensor.matmul(
                            s_ps[m][:],
                            lhsT=f8q[:, 2 * kl : 2 * kl + 2, m * P : (m + 1) * P],
                            rhs=f8q[:, 2 * kl : 2 * kl + 2, :],
                            start=(q == 0 and kl == 0),
                            stop=(q == NQ - 1 and kl == KQP - 1),
                            perf_mode=DR,
                        )
                    if q == NQ - 1:
                        # negated row-min of this S block (DVE)
                        mn = soft_p.tile([P, 1], f32, tag="mn", name=f"mn{b}{m}")
                        nc.vector.tensor_reduce(
                            out=mn[:], in_=s_ps[m][:], axis=AX.X, op=OP.min,
                            negate=True,
                        )
                        mns.append(mn)
                        if m == 1:
                            mx01 = soft_p.tile([P, 1], f32, tag="mn", name=f"mx01_{b}")
                            nc.vector.tensor_tensor(
                                out=mx01[:], in0=mns[0][:], in1=mns[1][:], op=OP.max
                            )
                            mns.append(mx01)  # mns[4]
                        elif m == 3:
                            mx23 = soft_p.tile([P, 1], f32, tag="mn", name=f"mx23_{b}")
                            nc.vector.tensor_tensor(
                                out=mx23[:], in0=mns[2][:], in1=mns[3][:], op=OP.max
                            )
                            mxall = soft_p.tile([P, 1], f32, tag="mn", name=f"mxa_{b}")
                            nc.vector.tensor_tensor(
                                out=mxall[:], in0=mns[4][:], in1=mx23[:], op=OP.max
                            )
                            mns.append(mxall)  # mns[5]

            # ---- finish the per-batch shift: all-reduce + sign/margin ----
            def emit_g(b, mns):
                gneg = soft_p.tile([P, 1], f32, tag="mn", name=f"gneg_{b}")
                nc.gpsimd.partition_all_reduce(
                    gneg[:], mns[5][:], channels=P, reduce_op=bass_isa.ReduceOp.max
                )
                g_sb = soft_p.tile([P, 1], f32, tag="mn", name=f"g_sb_{b}")
                nc.vector.tensor_scalar(
                    out=g_sb[:], in0=gneg[:], scalar1=-1.0, scalar2=GMARGIN,
                    op0=OP.mult, op1=OP.add,
                )
                return g_sb

            # ---- E = exp(g - S) bf16 (symmetric => fc lhsT strips),
            #      Z row sums, epilogue scale beta/Z (clamped) ----
            def emit_exps(b, s_ps, g_sb):
                es, brs = [], []
                for m in range(CT):
                    e = ee_p.tile([P, F], bf16, tag="e", name=f"e{b}{m}")
                    z = soft_p.tile([P, 1], f32, tag="mn", name=f"z{b}{m}")
                    nc.scalar.activation(
                        out=e[:],
                        in_=s_ps[m][:],
                        func=AF.Exp,
                        bias=g_sb[:],
                        scale=-1.0,
                        accum_out=z[:],
                    )
                    es.append(e)
                    zc = soft_p.tile([P, 1], f32, tag="mn", name=f"zc{b}{m}")
                    nc.vector.tensor_scalar_max(zc[:], z[:], 1e-37)
                    zr = soft_p.tile([P, 1], f32, tag="mn", name=f"zr{b}{m}")
                    nc.vector.reciprocal(zr[:], zc[:])
                    br = soft_p.tile([P, 1], f32, tag="mn", name=f"br{b}{m}")
                    nc.vector.tensor_tensor(
                        out=br[:], in0=beta_sb[:], in1=zr[:], op=OP.mult
                    )
                    brs.append(br)
                return es, brs

            # ---- one fc j-chunk: 16 bf16 matmuls + 4 fused epilogues
            #      + store; rhs/residual read the resident xb tiles ----
            def emit_fc_j(b, j, xbs, es, brs, store_dep, defer=None):
                h, jj = j // 4, j // 2
                jo = (j % 4) * F
                f_all = [
                    ps_fc.tile([P, F], f32, tag="fc", name=f"f_ps_{b}_{j}_{m}")
                    for m in range(CT)
                ]
                for kt in range(CT):
                    rhs = xbs[(kt, h)][:, jo : jo + F]
                    for m in range(CT):
                        nc.tensor.matmul(
                            f_all[m][:],
                            lhsT=es[kt][:, m * P : (m + 1) * P],
                            rhs=rhs,
                            start=(kt == 0),
                            stop=(kt == CT - 1),
                        )
                for m in range(CT):
                    f_ps = f_all[m]
                    ot = out_p.tile([P, F], f32, tag="out", name=f"ot_{b}_{j}_{m}")
                    # y = (beta/Z)[c] * fc_raw + x, one fused DVE op; stores
                    # go out per (j, m) at [128,512] so the final store tail
                    # drains as each epilogue lands instead of per j-pair
                    nc.vector.scalar_tensor_tensor(
                        out=ot[:],
                        in0=f_ps[:],
                        scalar=brs[m][:],
                        in1=xbs[(m, h)][:, jo : jo + F],
                        op0=OP.mult,
                        op1=OP.add,
                    )
                    del f_ps
                    if defer is not None:
                        defer.append((b, j, m, ot))
                    else:
                        sti = sp_eng.dma_start(
                            out=y_d[b, m * P : (m + 1) * P, j * F : (j + 1) * F],
                            in_=ot[:],
                        )
                        add_dep_helper(sti.ins, store_dep, reason="seg order")

            # ================= program =================
            xb0, xb1 = {}, {}
            f1t0, f1t1 = [], []
            s0 = [ps_s.tile([P, F], f32, tag="s", name=f"s_ps_0_{m}") for m in range(CT)]
            mns0, mns1 = [], []

            emit_loads(0, xb0)
            emit_xposes(0, xb0, f1t0)
            emit_repacks_dve(f1t0)
            for q in range(NQ):
                emit_S_chunk(0, q, s0, f1t0, mns0)
            g0 = emit_g(0, mns0)

            es0, brs0 = emit_exps(0, s0, g0)

            dep_l1 = seg_tail[0]
            pending = []
            emit_loads(1, xb1, halves=(0,), dep=dep_l1)
            for j in (0, 1):
                emit_fc_j(0, j, xb0, es0, brs0, None, defer=pending)
            emit_loads(1, xb1, dep=dep_l1, tiles=[(1, 0), (1, 1)],
                       xb_eng=nc.vector.tensor_copy)
            for j in (2, 3):
                emit_fc_j(0, j, xb0, es0, brs0, None, defer=pending)
            emit_loads(1, xb1, dep=dep_l1, tiles=[(1, 2), (1, 3)],
                       xb_eng=nc.vector.tensor_copy)
            for j in (4, 5):
                emit_fc_j(0, j, xb0, es0, brs0, None, defer=pending)
            emit_xposes(1, xb1, f1t1)
            emit_repacks_dve(f1t1)
            store_dep = seg_tail[0]  # stores fan on the last xpose
            for b_, j_, m_, ot_ in pending:
                sti = sp_eng.dma_start(
                    out=y_d[b_, m_ * P : (m_ + 1) * P, j_ * F : (j_ + 1) * F],
                    in_=ot_[:],
                )
                add_dep_helper(sti.ins, store_dep, reason="seg order")
            # S(1) and its exps go ahead of fc(0)'s last two j-chunks on
            # the PE so fc(1) can start immediately after fc(0) ends
            s1 = [ps_s.tile([P, F], f32, tag="s", name=f"s_ps_1_{m}") for m in range(CT)]
            for q in range(NQ):
                emit_S_chunk(1, q, s1, f1t1, mns1)
            g1 = emit_g(1, mns1)
            es1, brs1 = emit_exps(1, s1, g1)
            for j in (6, 7):
                emit_fc_j(0, j, xb0, es0, brs0, store_dep)
            for j in range(NCH):
                emit_fc_j(1, j, xb1, es1, brs1, store_dep)

    nc.finalize()
    return nc


def _get_nc():
    if "nc" not in _CACHE:
        _CACHE["nc"] = _build()
    return _CACHE["nc"]


def kernel(x: np.ndarray, beta: np.ndarray, **kw) -> np.ndarray:
    from concourse.bass_utils import run_bass_kernel_spmd

    x = np.ascontiguousarray(np.asarray(x, dtype=np.float32))
    beta = np.ascontiguousarray(np.asarray(beta, dtype=np.float32))
    assert x.shape == (B, C, 64, 64), x.shape

    xr = x.reshape(B, C, HW)
    in_maps = [
        {"x": np.ascontiguousarray(xr[i * BL : (i + 1) * BL]), "beta": beta}
        for i in range(NCORES)
    ]
    nc = _get_nc()
    res = run_bass_kernel_spmd(nc, in_maps, core_ids=list(range(NCORES)))
    out = np.concatenate([r["y"] for r in res.results], axis=0)
    return out.reshape(B, C, 64, 64).astype(np.float32)


# revision 37
# speedup vs baseline: 1.3163x; 1.0036x over previous
"""Channel-attention module (CAM) forward for Trainium2.

Computes, per batch b:
    f1 = x[b].reshape(C, H*W)                      # [512, 4096]
    S  = f1 @ f1.T                                 # [512, 512] (symmetric)
    G  = softmax(S_max - S, axis=-1)               # == exp(shift - S) / rowsum
    fc = G @ f1
    y[b] = beta * fc + x[b]

Sharding: data-parallel over batch B=16 across 8 NeuronCores (2 batches/core),
no cross-core communication.

Algorithmic structure (tuned against the Tile cost-model timeline, which the
grader reports):

* Softmax shift invariance + symmetry of S: a single per-batch scalar shift
  g = min(S) + margin makes E = exp(g - S) symmetric, so the exp output of S
  psum row-block kt — laid out [d-part, m-free] — IS the fc lhsT strip
  (unnormalized G^T). No PE transposes, no psum->sbuf copies, no activation
  table thrash. The per-row 1/Z normalization (Z from the exp accum_out,
  clamped so an underflowed row yields 0, never NaN) folds into the fused
  epilogue y = (beta/Z)[c]*fc_raw + x. g is computed on-chip: DVE negated
  row-mins per block (emitted inside the last S quarter), one GPSIMD
  partition_all_reduce(max) which also broadcasts, one DVE tensor_scalar.

* S runs in fp8(e4m3) with MatmulPerfMode.DoubleRow (2 contraction rows per
  partition per pass, 0.5 cycles/row): 4x fewer PE cycles than bf16. f1^T is
  produced by DMA-xbar-transposing the fp8 data with adjacent-n PAIRS packed
  as uint16 elements — half the transpose bytes of bf16. Dual-row Ldweights
  requires plane-major contiguous columns, so the pair-interleaved transpose
  output is repacked by two strided byte-gather copies per quarter (any
  consistent relabeling of the contraction rows is valid); slot0 on DVE,
  slot1 on Pool. fc stays bf16 (E's dynamic range needs it).

* x is resident as bf16 (xb) for the fc rhs and the residual (rel err ~2e-3,
  well under the 2e-2 gate); the fp32 staging tiles recycle immediately.
  Casts are spread so no in-order engine falls behind the load stream:
  xb on ACT (b1-h1 group on DVE), fp8 low halves on ACT, high halves on Pool.

DMA layout: the cost model serializes ALL DMA (SWDGE+HWDGE copies and xbar
transposes) on one device; a DmaTranspose<->DMACopy transition or an
inter-DMA dependency costs ~2.2-2.4us dead time, while FIFO order inside a
queue is free. The stream is organized as segments
  [loads b0][xposes b0][loads b1][xposes b1][stores b0][stores b1]
with fan-out deps (every DMA of a segment depends on the last DMA of the
previous one; some edges get pruned as transitively-implied and leak, which
is tolerated — leaked transposes fill load-stall gaps). Loads+stores ride
SP/HWDGE, transposes issue from ACT. fc(0)'s stores are deferred until the
b1 xpose segment is emitted so they fan on it. On the PE, S(1)+exps(1) are
emitted ahead of fc(0)'s last two j-chunks so fc(1) rolls on immediately.
Stores go out per (j, m) at [128,512] bf16 so the tail drains as each
epilogue lands (the bf16 output halves store bytes; kernel() upcasts on
the host). With the store tail off the critical path, the kernel ends on
the fc(1) epilogue chain at ~118us.
"""

import numpy as np

B, C, HW = 16, 512, 4096
NCORES = 8
BL = B // NCORES  # batches per core
P = 128
CT = C // P       # 4 c-tiles of 128 channels
NCH = 8           # n-chunks of 512
F = 512           # n-chunk size (psum free dim)
NQ = 4            # quarters of the n axis (transpose/S granularity)
QW = HW // NQ     # 1024
KQ = QW // P      # 8 k-tiles per quarter
KQP = KQ // 2     # 4 k-PAIR-tiles per quarter (fp8 DoubleRow: 256 n per tile)
HALF = HW // 2    # 2048, the load/cast granularity
GMARGIN = 75.0    # exp(g - S) with g = min(S) + GMARGIN: top entries reach
                  # e^75 and fc_raw stays < fp32 max; rows whose min is more
                  # than ~162 above the global min underflow to 0 and are
                  # clamped (their Z=0 -> beta/Z := 0, never NaN)

_CACHE = {}


def _build():
    import concourse.bass as bass  # noqa: F401
    import concourse.bass_isa as bass_isa
    import concourse.mybir as mybir
    import concourse.tile as tile
    from concourse import bacc

    f32 = mybir.dt.float32
    bf16 = mybir.dt.bfloat16
    f8 = mybir.dt.float8e4
    u16 = mybir.dt.uint16
    DR = mybir.MatmulPerfMode.DoubleRow
    AF = mybir.ActivationFunctionType
    OP = mybir.AluOpType
    AX = mybir.AxisListType
    ET = mybir.EngineType

    # Bacc (not plain Bass): its compile() runs generate_event_semaphores,
    # which splits excess sync waits onto EventSemaphore instructions —
    # required because TRN2 instructions (notably DMA_DIRECT2D_XPOSE) carry
    # at most one wait.
    nc = bacc.Bacc("TRN2", target_bir_lowering=False, debug=False)
    x_d = nc.dram_tensor("x", [BL, C, HW], f32, kind="ExternalInput")
    beta_d = nc.dram_tensor("beta", [1], f32, kind="ExternalInput")
    # y stored as bf16: the result is already bf16-accuracy-bound (the
    # residual x is bf16-resident), and halving the store bytes moves the
    # kernel off the serial-DMA store tail; kernel() upcasts on the host
    y_d = nc.dram_tensor("y", [BL, C, HW], bf16, kind="ExternalOutput")

    sp_eng = nc.engines[ET.SP]

    from bass_rust import add_dep_helper

    with tile.TileContext(nc) as tc:
        with (
            tc.tile_pool(name="singles", bufs=1) as singles,
            tc.tile_pool(name="xf", bufs=4) as xf,           # [128,2048] f32 staging
            tc.tile_pool(name="xb", bufs=17) as xb_p,        # [128,2048] bf16 resident
            tc.tile_pool(name="x8", bufs=8) as x8_p,        # [128,2048] fp8 staging
            tc.tile_pool(name="f1t", bufs=5) as f1t_p,       # [128,4,512] u16 (fp8 pairs)
            tc.tile_pool(name="f8t", bufs=5) as f8t_p,       # [128,8,512] fp8 plane-major
            tc.tile_pool(name="ee", bufs=6) as ee_p,         # [128,512] bf16
            tc.tile_pool(name="soft", bufs=24) as soft_p,    # [128,1] f32
            tc.tile_pool(name="outs", bufs=22) as out_p,     # [128,512] bf16
            tc.tile_pool(name="ps_s", bufs=4, space="PSUM") as ps_s,
            tc.tile_pool(name="ps_fc", bufs=4, space="PSUM") as ps_fc,
        ):
            beta_sb = singles.tile([P, 1], f32)
            nc.gpsimd.dma_start(out=beta_sb[:], in_=beta_d[:].to_broadcast([P, 1]))

            # last DMA instruction of the previous stream segment: every DMA
            # of the current segment fan-out-depends on it
            seg_tail = [None]
            last_xf = [None]

            def emit_loads(b, xbs, halves=(0, 1), dep="prev", tiles=None, xb_eng=None):
                prev_tail = seg_tail[0] if dep == "prev" else dep
                for h, ct in (tiles or [(h, ct) for h in halves for ct in range(CT)]):
                    if True:
                        t = xf.tile([P, HALF], f32, tag="xf", name=f"x_{b}_{ct}_{h}")
                        ld = sp_eng.dma_start(
                            out=t[:],
                            in_=x_d[b, ct * P : (ct + 1) * P, h * HALF : (h + 1) * HALF],
                        )
                        if prev_tail is not None:
                            add_dep_helper(ld.ins, prev_tail, reason="seg order")
                        # resident bf16 copy (fc rhs + residual) and a
                        # transient fp8 copy (S transpose feed); the fp32
                        # staging tile then recycles. Casts are spread over
                        # ACT/Pool (and DVE for the b1-h1 group) so no single
                        # in-order engine falls behind the load stream.
                        xbt = xb_p.tile([P, HALF], bf16, tag="xb", name=f"xb_{b}_{ct}_{h}")
                        if xb_eng is None:
                            nc.scalar.copy(out=xbt[:], in_=t[:])
                        else:
                            xb_eng(out=xbt[:], in_=t[:])
                        x8t = x8_p.tile([P, HALF], f8, tag="x8", name=f"x8_{b}_{ct}_{h}")
                        # fp8 halves split ACT/Pool: together with the xb cast
                        # each engine stays just under the load cadence, and
                        # DVE stays free for the fc epilogues
                        nc.scalar.copy(out=x8t[:, 0:QW], in_=t[:, 0:QW])
                        nc.gpsimd.tensor_copy(out=x8t[:, QW : 2 * QW], in_=t[:, QW : 2 * QW])
                        xbs[(ct, h)] = xbt
                        xbs[("x8", ct, h)] = x8t
                        seg_tail[0] = ld.ins
                        last_xf[0] = t

            # xbar transposes (ACT/HWDGE) of the fp8 tiles, with adjacent-n
            # fp8 PAIRS packed as uint16 elements: half the transpose bytes,
            # and the pair-interleaved layout is exactly what the DoubleRow
            # matmul wants. Quarter-major so S's chunks unblock progressively.
            def emit_xposes(b, xbs, f1ts, quarters=tuple(range(NQ)), slot1=None):
                prev_tail = seg_tail[0]
                for q in quarters:
                    fq = f1t_p.tile([P, KQP, F], u16, tag="f1t", name=f"f1t_{b}_{q}")
                    h, qo = q // 2, (q % 2) * (QW // 2)
                    for ct in range(CT):
                        # MUST issue from ACT: SP-issued dma_start_transpose
                        # reproducibly faults the device (the known TC5 hang
                        # that moved hwdge transposes off SP upstream)
                        xp = nc.scalar.dma_start_transpose(
                            fq[:, :, ct * P : (ct + 1) * P],
                            xbs[("x8", ct, h)][:].bitcast(u16)[:, qo : qo + QW // 2],
                        )
                        add_dep_helper(xp.ins, prev_tail, reason="seg order")
                        seg_tail[0] = xp.ins
                    # repack pair-interleaved fp8 into plane-major k-tiles
                    # (dual-row Ldweights requires contiguous per-plane
                    # columns; any consistent k-relabeling is valid for the
                    # contraction). Slot 1 on Pool here; slot 0 on DVE via
                    # emit_repacks_dve, placed where DVE has slack.
                    f8q = f8t_p.tile([P, KQ, F], f8, tag="f8t", name=f"f8t_{b}_{q}")
                    fq8 = fq[:].bitcast(f8)  # [128, KQP, 2F]
                    # slot-1 repack engine: ACT for b0 (idle in that window,
                    # and Pool's 0.6-efficiency pace would gate S(0)); Pool
                    # for b1 (ACT is busy with the b1 casts then)
                    (slot1 or nc.gpsimd.tensor_copy)(
                        out=f8q[:, 1::2, :], in_=fq8[:, :, 1::2]
                    )
                    f1ts.append((fq, f8q))

            def emit_repacks_dve(f1ts):
                for fq, f8q in f1ts:
                    nc.vector.tensor_copy(
                        out=f8q[:, 0::2, :], in_=fq[:].bitcast(f8)[:, :, 0::2]
                    )

            def emit_repacks_dve_slot1(f1ts):
                for fq, f8q in f1ts:
                    nc.vector.tensor_copy(
                        out=f8q[:, 1::2, :], in_=fq[:].bitcast(f8)[:, :, 1::2]
                    )

            # ---- one quarter-chunk of S = f1 @ f1^T (16 matmuls) ----
            # In the last quarter the per-block row stats + the g combine
            # tree are emitted m-by-m so they hide under remaining matmuls.
            def emit_S_chunk(b, q, s_ps, f1ts, mns):
                for m in range(CT):
                    for kl in range(KQP):
                        f8q = f1ts[q][1]
                        nc.tensor.matmul(
                            s_ps[m][:],
                            lhsT=f8q[:, 2 * kl : 2 * kl + 2, m * P : (m + 1) * P],
                            rhs=f8q[:, 2 * kl : 2 * kl + 2, :],
                            start=(q == 0 and kl == 0),
                            stop=(q == NQ - 1 and kl == KQP - 1),
                            perf_mode=DR,
                        )
                    if q == NQ - 1:
                        # negated row-min of this S block (DVE)
                        mn = soft_p.tile([P, 1], f32, tag="mn", name=f"mn{b}{m}")
                        nc.vector.tensor_reduce(
                            out=mn[:], in_=s_ps[m][:], axis=AX.X, op=OP.min,
                            negate=True,
                        )
                        mns.append(mn)
                        if m == 1:
                            mx01 = soft_p.tile([P, 1], f32, tag="mn", name=f"mx01_{b}")
                            nc.vector.tensor_tensor(
                                out=mx01[:], in0=mns[0][:], in1=mns[1][:], op=OP.max
                            )
                            mns.append(mx01)  # mns[4]
                        elif m == 3:
                            mx23 = soft_p.tile([P, 1], f32, tag="mn", name=f"mx23_{b}")
                            nc.vector.tensor_tensor(
                                out=mx23[:], in0=mns[2][:], in1=mns[3][:], op=OP.max
                            )
                            mxall = soft_p.tile([P, 1], f32, tag="mn", name=f"mxa_{b}")
                            nc.vector.tensor_tensor(
                                out=mxall[:], in0=mns[4][:], in1=mx23[:], op=OP.max
                            )
                            mns.append(mxall)  # mns[5]

            # ---- finish the per-batch shift: all-reduce + sign/margin ----
            def emit_g(b, mns):
                gneg = soft_p.tile([P, 1], f32, tag="mn", name=f"gneg_{b}")
                nc.gpsimd.partition_all_reduce(
                    gneg[:], mns[5][:], channels=P, reduce_op=bass_isa.ReduceOp.max
                )
                g_sb = soft_p.tile([P, 1], f32, tag="mn", name=f"g_sb_{b}")
                nc.vector.tensor_scalar(
                    out=g_sb[:], in0=gneg[:], scalar1=-1.0, scalar2=GMARGIN,
                    op0=OP.mult, op1=OP.add,
                )
                return g_sb

            # ---- E = exp(g - S) bf16 (symmetric => fc lhsT strips),
            #      Z row sums, epilogue scale beta/Z (clamped) ----
            def emit_exps(b, s_ps, g_sb):
                es, brs = [], []
                for m in range(CT):
                    e = ee_p.tile([P, F], bf16, tag="e", name=f"e{b}{m}")
                    z = soft_p.tile([P, 1], f32, tag="mn", name=f"z{b}{m}")
                    nc.scalar.activation(
                        out=e[:],
                        in_=s_ps[m][:],
                        func=AF.Exp,
                        bias=g_sb[:],
                        scale=-1.0,
                        accum_out=z[:],
                    )
                    es.append(e)
                    zc = soft_p.tile([P, 1], f32, tag="mn", name=f"zc{b}{m}")
                    nc.vector.tensor_scalar_max(zc[:], z[:], 1e-37)
                    zr = soft_p.tile([P, 1], f32, tag="mn", name=f"zr{b}{m}")
                    nc.vector.reciprocal(zr[:], zc[:])
                    br = soft_p.tile([P, 1], f32, tag="mn", name=f"br{b}{m}")
                    nc.vector.tensor_tensor(
                        out=br[:], in0=beta_sb[:], in1=zr[:], op=OP.mult
                    )
                    brs.append(br)
                return es, brs

            # ---- one fc j-chunk: 16 bf16 matmuls + 4 fused epilogues
            #      + store; rhs/residual read the resident xb tiles ----
            def emit_fc_j(b, j, xbs, es, brs, store_dep, defer=None):
                h, jj = j // 4, j // 2
                jo = (j % 4) * F
                f_all = [
                    ps_fc.tile([P, F], f32, tag="fc", name=f"f_ps_{b}_{j}_{m}")
                    for m in range(CT)
                ]
                for kt in range(CT):
                    rhs = xbs[(kt, h)][:, jo : jo + F]
                    for m in range(CT):
                        nc.tensor.matmul(
                            f_all[m][:],
                            lhsT=es[kt][:, m * P : (m + 1) * P],
                            rhs=rhs,
                            start=(kt == 0),
                            stop=(kt == CT - 1),
                        )
                for m in range(CT):
                    f_ps = f_all[m]
                    ot = out_p.tile([P, F], bf16, tag="out", name=f"ot_{b}_{j}_{m}")
                    # y = (beta/Z)[c] * fc_raw + x, one fused DVE op; stores
                    # go out per (j, m) at [128,512] so the final store tail
                    # drains as each epilogue lands instead of per j-pair
                    nc.vector.scalar_tensor_tensor(
                        out=ot[:],
                        in0=f_ps[:],
                        scalar=brs[m][:],
                        in1=xbs[(m, h)][:, jo : jo + F],
                        op0=OP.mult,
                        op1=OP.add,
                    )
                    del f_ps
                    if defer is not None:
                        defer.append((b, j, m, ot))
                    else:
                        sti = sp_eng.dma_start(
                            out=y_d[b, m * P : (m + 1) * P, j * F : (j + 1) * F],
                            in_=ot[:],
                        )
                        add_dep_helper(sti.ins, store_dep, reason="seg order")

            # ================= program =================
            xb0, xb1 = {}, {}
            f1t0, f1t1 = [], []
            s0 = [ps_s.tile([P, F], f32, tag="s", name=f"s_ps_0_{m}") for m in range(CT)]
            mns0, mns1 = [], []

            emit_loads(0, xb0)
            emit_xposes(0, xb0, f1t0)
            emit_repacks_dve(f1t0)
            for q in range(NQ):
                emit_S_chunk(0, q, s0, f1t0, mns0)
            g0 = emit_g(0, mns0)

            es0, brs0 = emit_exps(0, s0, g0)

            dep_l1 = seg_tail[0]
            pending = []
            emit_loads(1, xb1, halves=(0,), dep=dep_l1)
            for j in (0, 1):
                emit_fc_j(0, j, xb0, es0, brs0, None, defer=pending)
            emit_loads(1, xb1, dep=dep_l1, tiles=[(1, 0), (1, 1)],
                       xb_eng=nc.vector.tensor_copy)
            for j in (2, 3):
                emit_fc_j(0, j, xb0, es0, brs0, None, defer=pending)
            emit_loads(1, xb1, dep=dep_l1, tiles=[(1, 2), (1, 3)],
                       xb_eng=nc.vector.tensor_copy)
            for j in (4, 5):
                emit_fc_j(0, j, xb0, es0, brs0, None, defer=pending)
            emit_xposes(1, xb1, f1t1)
            emit_repacks_dve(f1t1)
            store_dep = seg_tail[0]  # stores fan on the last xpose
            for b_, j_, m_, ot_ in pending:
                sti = sp_eng.dma_start(
                    out=y_d[b_, m_ * P : (m_ + 1) * P, j_ * F : (j_ + 1) * F],
                    in_=ot_[:],
                )
                add_dep_helper(sti.ins, store_dep, reason="seg order")
            # S(1) and its exps go ahead of fc(0)'s last two j-chunks on
            # the PE so fc(1) can start immediately after fc(0) ends
            s1 = [ps_s.tile([P, F], f32, tag="s", name=f"s_ps_1_{m}") for m in range(CT)]
            for q in range(NQ):
                emit_S_chunk(1, q, s1, f1t1, mns1)
            g1 = emit_g(1, mns1)
            es1, brs1 = emit_exps(1, s1, g1)
            for j in (6, 7):
                emit_fc_j(0, j, xb0, es0, brs0, store_dep)
            for j in range(NCH):
                emit_fc_j(1, j, xb1, es1, brs1, store_dep)

    nc.finalize()
    return nc


def _get_nc():
    if "nc" not in _CACHE:
        _CACHE["nc"] = _build()
    return _CACHE["nc"]


def kernel(x: np.ndarray, beta: np.ndarray, **kw) -> np.ndarray:
    from concourse.bass_utils import run_bass_kernel_spmd

    x = np.ascontiguousarray(np.asarray(x, dtype=np.float32))
    beta = np.ascontiguousarray(np.asarray(beta, dtype=np.float32))
    assert x.shape == (B, C, 64, 64), x.shape

    xr = x.reshape(B, C, HW)
    in_maps = [
        {"x": np.ascontiguousarray(xr[i * BL : (i + 1) * BL]), "beta": beta}
        for i in range(NCORES)
    ]
    nc = _get_nc()
    res = run_bass_kernel_spmd(nc, in_maps, core_ids=list(range(NCORES)))
    out = np.concatenate([r["y"] for r in res.results], axis=0)
    return out.reshape(B, C, 64, 64).astype(np.float32)
